# revision 1
# baseline (speedup 1.0000x reference)
"""Trainium2 Bass kernel for nn_AttentionOnDetail (sparse patch attention).

Data-parallel over batch B=8 across 8 NeuronCores; one batch per core.
Per core:
  phase 1: stream x[b] (4MB) in patch-major tiles [128 patches, 2048];
           per-patch sum-of-squares via ScalarE activation(Square,
           accum_out) and dot(patch, patch_w) via VectorE
           tensor_tensor_reduce -> 512 logits.
  top-4:   top-8 values -> 4th value threshold -> mask * (512-i) ->
           max_index returns the 4 selected patch ids ascending;
           expand to 64 token ids; indirect DMA gathers x_sel.
  phase 2: qkvg projection of only the 64 selected tokens (the
           reference computes all 8192), DRAM-bounce rearrange into
           q/k/v/g, RoPE + rmsnorm + tao, causal attention over
           65 rows (sink + 64), sigmoid gating, output projection.
"""

import sys
import numpy as np

for _p in ("/opt/trn_rl_repo",):
    if _p not in sys.path:
        sys.path.insert(0, _p)

import concourse.bass as bass
import concourse.bacc as bacc
import concourse.tile as tile
from concourse import mybir
from concourse.bass_utils import run_bass_kernel_spmd

F32 = mybir.dt.float32
I32 = mybir.dt.int32
U32 = mybir.dt.uint32
AF = mybir.ActivationFunctionType
ALU = mybir.AluOpType
AX = mybir.AxisListType

B, T, C, H, T0 = 8, 8192, 128, 8, 16
NP = T // T0          # 512 patches
PATCH = T0 * C        # 2048 elements per patch
S = 65                # sink + 64 selected tokens
NSEL = 64
FQ = 4 * C * H        # 4096
EPS = 1.1920929e-07
SCALE = 1.0 / float(np.sqrt(np.float32(C)))
NEG_BIG = -1.0e30


def rap(t, apl, offset=0):
    """Raw AP over a tile/AP's storage, flat element strides.

    For SBUF tensors the partition step of dim0 equals the tensor's
    free size per partition.
    """
    base = t if isinstance(t, bass.AP) else t[:]
    return bass.AP(tensor=base.tensor, offset=base.offset + offset,
                   ap=[list(x) for x in apl])


def build_kernel(nc):
    xb = nc.dram_tensor("xb", [T, C], F32, kind="ExternalInput")
    pw = nc.dram_tensor("pw", [PATCH], F32, kind="ExternalInput")
    wqkvg = nc.dram_tensor("wqkvg", [FQ, C], F32, kind="ExternalInput")
    wout = nc.dram_tensor("wout", [C, H * C], F32, kind="ExternalInput")
    sink = nc.dram_tensor("sink", [H, C], F32, kind="ExternalInput")
    cosd = nc.dram_tensor("cosd", [S, 64], F32, kind="ExternalInput")
    sind = nc.dram_tensor("sind", [S, 64], F32, kind="ExternalInput")
    tao = nc.dram_tensor("tao", [S, 2], F32, kind="ExternalInput")
    ident = nc.dram_tensor("ident", [128, 128], F32, kind="ExternalInput")
    off16 = nc.dram_tensor("off16", [NSEL, 1], F32, kind="ExternalInput")
    negio = nc.dram_tensor("negio", [1, NP], F32, kind="ExternalInput")
    cmask = nc.dram_tensor("cmask", [S, S], F32, kind="ExternalInput")
    repmat = nc.dram_tensor("repmat", [4, NSEL], F32, kind="ExternalInput")
    out = nc.dram_tensor("out", [NSEL, C], F32, kind="ExternalOutput")

    with tile.TileContext(nc) as tc:
        _emit(tc, nc, xb, pw, wqkvg, wout, sink, cosd, sind, tao, ident,
              off16, negio, cmask, repmat, out)
    return nc


def _emit(tc, nc, xb, pw, wqkvg, wout, sink, cosd, sind, tao, ident,
          off16, negio, cmask, repmat, out):
    import os
    LEVEL = int(os.environ.get("KLEVEL", "9"))
    from contextlib import ExitStack
    ctx = ExitStack()
    with ctx:
        const1 = ctx.enter_context(tc.tile_pool(name="const1", bufs=1))
        xpool = ctx.enter_context(tc.tile_pool(name="xpool", bufs=3))
        junkp = ctx.enter_context(tc.tile_pool(name="junkp", bufs=1))
        stat = ctx.enter_context(tc.tile_pool(name="stat", bufs=4))
        sb = ctx.enter_context(tc.tile_pool(name="sb", bufs=1))
        psall = ctx.enter_context(tc.tile_pool(name="psall", bufs=1,
                                                space="PSUM"))
        # one tile owning all 8 PSUM banks; regions are choreographed
        # manually (same-tile writes avoid slot-realloc wait explosions)
        PS = psall.tile([128, 4096], F32)
        # bank layout (f32 offsets):
        #   0:512     transpose slot A
        #   512:1024  transpose slot B
        #   1024:2048 qkvg matmul groups
        #   2048:2560 logits-T / repmat / x_selT / out
        #   2560:3584 att / y
        #   3584:4096 junk absorber columns
        dramp = ctx.enter_context(tc.tile_pool(name="dramp", bufs=1,
                                               space="DRAM"))
        # DRAM scratch: natural qkvg copy [64 tokens, 4096], then
        # per-tensor row-permuted copy [4, 64, 1024]
        qperm = dramp.tile([NSEL, FQ], F32)
        qperm2 = dramp.tile([4, S, H * C], F32)

        # ---------------- constants / weight prep ----------------
        ident_t = const1.tile([128, 128], F32)
        nc.sync.dma_start(out=ident_t[:, :], in_=ident[:, :])

        # pw broadcast to 128 partitions via K=1 matmul (SWDGE step-0
        # partition-broadcast DMA crashes the exec unit)
        pw_sb = const1.tile([1, PATCH], F32)
        nc.sync.dma_start(out=pw_sb[:, :], in_=rap(pw[:], [[1, 1], [1, PATCH]]))
        ones_t = const1.tile([1, 128], F32)
        nc.vector.memset(ones_t[:, :], 1.0)
        pwB = const1.tile([128, PATCH], F32)

        # absorb the ident_t DMA wait (every PE matmul may carry at most
        # ONE sync wait -- walrus funnels waits into the LDW struct)
        nc.tensor.matmul(out=PS[:, 3584:3585], lhsT=ident_t[:, :],
                         rhs=ident_t[:, 0:1], start=True, stop=True)

        # absorb pw's DMA lane, then broadcast pw into 128 partitions
        nc.tensor.matmul(out=PS[0:128, 3585:3586], lhsT=pw_sb[0:1, 0:128],
                         rhs=pw_sb[0:1, 0:1], start=True, stop=True)
        for q4 in range(4):
            pbase = 1024 + 512 * (q4 % 2) if q4 < 2 else 2560 + 512 * (q4 % 2)
            nc.tensor.matmul(out=PS[:, pbase:pbase + 512],
                             lhsT=ones_t[:, :],
                             rhs=pw_sb[:, 512 * q4:512 * (q4 + 1)],
                             start=True, stop=True)
            nc.scalar.copy(out=pwB[:, 512 * q4:512 * (q4 + 1)],
                           in_=PS[:, pbase:pbase + 512])

        # W_qkvg natural: w_nat[p, t, c] = W[t*128+p, c]
        w_nat = const1.tile([128, 32, C], F32)
        nc.sync.dma_start(
            out=w_nat[:, :, :],
            in_=rap(wqkvg[:, :], [[C, 128], [128 * C, 32], [1, C]]))
        # wqT[:, t, :] = W[t*128:(t+1)*128, :].T   (c-major)
        wqT = const1.tile([128, 32, C], F32)
        for g in range(8):
            base = 512 * (g % 2)
            for j in range(4):
                nc.tensor.matmul(
                    out=PS[:, base + j * 128:base + (j + 1) * 128],
                    lhsT=w_nat[:, 4 * g + j, :], rhs=ident_t[:, :],
                    start=True, stop=True)
            nc.vector.tensor_copy(
                out=wqT[:, 4 * g:4 * g + 4, :],
                in_=PS[:, base:base + 512].rearrange(
                    "p (a b) -> p a b", a=4))

        wo_nat = const1.tile([128, H, C], F32)
        nc.sync.dma_start(
            out=wo_nat[:, :, :],
            in_=rap(wout[:, :], [[H * C, 128], [128, H], [1, C]]))
        # absorb wo_nat's DMA wait on PE
        nc.tensor.matmul(out=PS[:, 3586:3587], lhsT=wo_nat[:, 0, :],
                         rhs=ident_t[:, 0:1], start=True, stop=True)
        woT = const1.tile([128, H, C], F32)
        for g in range(2):
            base = 512 * (g % 2)
            for j in range(4):
                nc.tensor.matmul(
                    out=PS[:, base + j * 128:base + (j + 1) * 128],
                    lhsT=wo_nat[:, 4 * g + j, :], rhs=ident_t[:, :],
                    start=True, stop=True)
            nc.vector.tensor_copy(
                out=woT[:, 4 * g:4 * g + 4, :],
                in_=PS[:, base:base + 512].rearrange(
                    "p (a b) -> p a b", a=4))

        eps_t = const1.tile([128, 1], F32)
        nc.vector.memset(eps_t[:, :], EPS)

        # seq-position permutation: partition p<64 = position p+1,
        # partition 64 = position 0 (sink)
        cos_t = const1.tile([S, 64], F32)
        nc.sync.dma_start(out=cos_t[0:NSEL, :], in_=cosd[1:S, :])
        nc.sync.dma_start(out=cos_t[NSEL:S, :], in_=cosd[0:1, :])
        sinD = const1.tile([S, 2, 64], F32)   # [:,0,:]=+sin  [:,1,:]=-sin
        nc.sync.dma_start(out=sinD[0:NSEL, 0, :], in_=sind[1:S, :])
        nc.sync.dma_start(out=sinD[NSEL:S, 0, :], in_=sind[0:1, :])
        nc.scalar.mul(out=sinD[0:NSEL, 1, :], in_=sinD[0:NSEL, 0, :],
                      mul=-1.0)
        nc.scalar.mul(out=sinD[NSEL:S, 1, :], in_=sinD[NSEL:S, 0, :],
                      mul=-1.0)

        taob = const1.tile([S, 2], F32)
        nc.sync.dma_start(out=taob[:, :], in_=tao[:, :])

        off16_t = const1.tile([NSEL, 1], F32)
        nc.sync.dma_start(out=off16_t[:, :], in_=off16[:, :])
        repmat_t = const1.tile([4, NSEL], F32)
        nc.sync.dma_start(out=repmat_t[:, :], in_=repmat[:, :])
        # absorb repmat_t's DMA wait on PE early
        nc.tensor.matmul(out=PS[0:NSEL, 3585:3586], lhsT=repmat_t[:, :],
                         rhs=repmat_t[:, 0:1], start=True, stop=True)
        negio_t = const1.tile([1, NP], F32)
        nc.sync.dma_start(out=negio_t[:, :], in_=negio[:, :])
        cmask_t = const1.tile([S, S], F32)
        nc.sync.dma_start(out=cmask_t[:, :], in_=cmask[:, :])

        # ---------------- phase 1: per-patch stats ----------------
        touch = const1.tile([128, 24], F32)
        nc.vector.tensor_copy(out=touch[:, 1:2], in_=pwB[:, 0:1])
        # absorb every constant table's DMA lane on DVE early (cheap,
        # off the critical path)
        nc.vector.tensor_copy(out=touch[0:NSEL, 2:3], in_=cos_t[0:NSEL, 0:1])
        nc.vector.tensor_copy(out=touch[NSEL:S, 3:4], in_=cos_t[NSEL:S, 0:1])
        nc.vector.tensor_copy(out=touch[0:NSEL, 4:5],
                              in_=sinD[0:NSEL, 0, 0:1])
        nc.vector.tensor_copy(out=touch[NSEL:S, 5:6],
                              in_=sinD[NSEL:S, 0, 0:1])
        nc.vector.tensor_copy(out=touch[0:S, 6:7], in_=cmask_t[:, 0:1])
        nc.vector.tensor_copy(out=touch[0:S, 7:8], in_=taob[:, 0:1])
        nc.vector.tensor_copy(out=touch[0:1, 8:9], in_=negio_t[:, 0:1])
        nc.vector.tensor_copy(out=touch[0:NSEL, 9:10], in_=off16_t[:, :])
        logits_col = stat.tile([128, 4], F32)
        for i in range(4):
            xp = xpool.tile([128, PATCH], F32, tag="xp")
            nc.sync.dma_start(
                out=xp[:, :],
                in_=rap(xb[:, :], [[PATCH, 128], [1, PATCH]],
                        offset=i * 128 * PATCH))
            junk = junkp.tile([128, PATCH], F32, tag="junk")
            ss = stat.tile([128, 1], F32, tag="ss")
            nc.scalar.activation(out=junk[:, :], in_=xp[:, :], func=AF.Square,
                                 accum_out=ss[:, :])
            junk2 = junkp.tile([128, PATCH], F32, tag="junk2")
            dotv = stat.tile([128, 1], F32, tag="dotv")
            nc.vector.scalar_tensor_tensor(
                out=junk2[:, :], in0=xp[:, :], scalar=1.0, in1=pwB[:, :],
                op0=ALU.mult, op1=ALU.mult, accum_out=dotv[:, :])
            sq = stat.tile([128, 1], F32, tag="sq")
            nc.scalar.activation(out=sq[:, :], in_=ss[:, :], func=AF.Sqrt,
                                 bias=eps_t[0:128, 0:1], scale=1.0 / PATCH)
            rs = stat.tile([128, 1], F32, tag="rs")
            nc.vector.reciprocal(out=rs[:, :], in_=sq[:, :])
            nc.vector.tensor_mul(logits_col[:, i:i + 1], dotv[:, :], rs[:, :])

        # one row [1, 512]: PE-transpose [128, 4] -> [4, 128], then a
        # contiguous SBUF->SBUF DMA into [1, 512]
        nc.tensor.matmul(out=PS[0:4, 2048:2176], lhsT=logits_col[:, :],
                         rhs=ident_t[:, :], start=True, stop=True)
        lrow4 = stat.tile([4, 128], F32)
        nc.scalar.copy(out=lrow4[:, :], in_=PS[0:4, 2048:2176])
        logits_row = stat.tile([1, NP], F32)
        nc.sync.dma_start(out=logits_row[:, :], in_=lrow4[:, :])

        if LEVEL == 1:
            nc.sync.dma_start(out=out[:, 0:4], in_=logits_col[0:64, :])
            return
        # ---------------- top-4 selection ----------------
        max8 = stat.tile([1, 8], F32)
        nc.vector.max(out=max8[:, :], in_=logits_row[:, :])
        mask = stat.tile([1, NP], F32)
        nc.vector.tensor_scalar(out=mask[:, :], in0=logits_row[:, :],
                                scalar1=max8[:, 3:4], scalar2=None,
                                op0=ALU.is_ge)
        masked = stat.tile([1, NP], F32)
        nc.vector.tensor_mul(masked[:, :], mask[:, :], negio_t[:, :])
        mm8 = stat.tile([1, 8], F32)
        nc.vector.max(out=mm8[:, :], in_=masked[:, :])
        idx8 = stat.tile([1, 8], U32)
        nc.vector.max_index(out=idx8[:, :], in_max=mm8[:, :],
                            in_values=masked[:, :])
        idxf = stat.tile([1, 8], F32)
        nc.vector.tensor_copy(out=idxf[:, :], in_=idx8[:, :])

        # token ids: move ids to a column via tiny DMA, then matmul with
        # the replication matrix repmat[k, m] = 16*(m//16 == k)
        idxc = stat.tile([4, 1], F32)
        nc.sync.dma_start(out=idxc[:, :], in_=idxf[0:1, 0:4])
        # absorb idxc's DMA wait
        nc.tensor.matmul(out=PS[0:1, 3587:3588], lhsT=idxc[:, :],
                         rhs=idxc[:, :], start=True, stop=True)
        nc.tensor.matmul(out=PS[0:NSEL, 2048:2049], lhsT=repmat_t[:, :],
                         rhs=idxc[:, :], start=True, stop=True)
        nc.vector.tensor_copy(out=touch[0:4, 10:11], in_=lrow4[:, 0:1])
        tok_f = stat.tile([NSEL, 1], F32)
        nc.vector.tensor_add(out=tok_f[:, :], in0=PS[0:NSEL, 2048:2049],
                             in1=off16_t[:, :])
        tok_i = stat.tile([NSEL, 1], I32)
        nc.vector.tensor_copy(out=tok_i[:, :], in_=tok_f[:, :])

        x_sel = sb.tile([NSEL, C], F32, tag="x_sel")
        nc.gpsimd.indirect_dma_start(
            out=x_sel[:, :], out_offset=None, in_=xb[:, :],
            in_offset=bass.IndirectOffsetOnAxis(ap=tok_i[:, 0:1], axis=0))

        if LEVEL == 2:
            nc.sync.dma_start(out=out[:, :], in_=x_sel[:, :])
            return
        # ---------------- qkvg projection (64 tokens) ----------------
        # absorb x_sel's (indirect) DMA wait
        nc.tensor.matmul(out=PS[:, 3588:3589], lhsT=x_sel[:, :],
                         rhs=ident_t[0:NSEL, 0:1], start=True, stop=True)
        nc.tensor.matmul(out=PS[:, 2048:2048 + NSEL], lhsT=x_sel[:, :],
                         rhs=ident_t[0:NSEL, 0:NSEL], start=True, stop=True)
        nc.scalar.copy(out=touch[0:NSEL, 11:12], in_=tok_f[:, :])
        x_selT = sb.tile([128, NSEL], F32, tag="x_selT")
        nc.scalar.copy(out=x_selT[:, :], in_=PS[:, 2048:2048 + NSEL])

        qkvg_sb = sb.tile([NSEL, FQ], F32, tag="qkvg")
        for grp in range(4):
            for j in range(2):
                k = grp * 2 + j
                nc.tensor.matmul(
                    out=PS[0:NSEL, 1024 + j * 512:1024 + (j + 1) * 512],
                    lhsT=x_selT[:, :],
                    rhs=wqT[:, 4 * k:4 * k + 4, :], start=True, stop=True)
            nc.scalar.copy(out=qkvg_sb[:, 1024 * grp:1024 * (grp + 1)],
                           in_=PS[0:NSEL, 1024:2048])

        # plain copy to DRAM; the q/k/v/g layout is an identity map in
        # flat bytes (token row 4096 = 4 dst rows of 1024)
        nc.sync.dma_start(out=qperm[:, :], in_=qkvg_sb[:, :])

        # q/k/v/g token-major [*, 8, 128] (contiguous reads)
        q_all = sb.tile([S, H, C], F32, tag="q_all")
        k_all = sb.tile([S, H, C], F32, tag="k_all")
        v_all = sb.tile([S, H, C], F32, tag="v_all")
        g_all = sb.tile([NSEL, H, C], F32, tag="g_all")
        tc.strict_bb_all_engine_barrier()
        # DRAM->DRAM row permutation into final order, sink appended
        qperm_v = qperm[:, :].rearrange("(a b) f -> a b f", b=16)
        for tens in range(4):
            joff = 4 * tens
            nc.sync.dma_start(
                out=qperm2[tens, 0:NSEL, :],
                in_=qperm_v[:, joff:joff + 4, :])
            if tens < 3:
                nc.sync.dma_start(
                    out=qperm2[tens, NSEL:S, :],
                    in_=rap(sink[:, :], [[0, 1], [1, H * C]]))
        tc.strict_bb_all_engine_barrier()
        # absorb the barrier semaphore on PE, DVE and ACT
        nc.tensor.matmul(out=PS[:, 3589:3590], lhsT=ident_t[:, :],
                         rhs=ident_t[:, 0:1], start=True, stop=True)
        nc.vector.tensor_copy(out=touch[:, 12:13], in_=eps_t[:, :])
        nc.scalar.copy(out=touch[0:1, 13:14], in_=eps_t[0:1, 0:1])
        # one contiguous readback per tensor (single DMA lane each)
        for tens, dst in enumerate((q_all, k_all, v_all, g_all)):
            ns = S if tens < 3 else NSEL
            nc.sync.dma_start(
                out=dst[0:ns, :, :],
                in_=qperm2[tens, 0:ns, :].rearrange("s (h c) -> s h c", h=H))

        if LEVEL == 3:
            nc.sync.dma_start(out=out[:, :], in_=q_all[0:NSEL, 0, :])
            return
        # ---------------- RoPE + rmsnorm + tao ----------------
        def rope_norm(src, dst, tao_col):
            r = sb.tile([S, H, C], F32, tag="rope_r")
            cos_b = cos_t[:, :].rearrange(
                "s (a b c2) -> s a b c2", a=1, b=1).to_broadcast([S, H, 2, 64])
            nc.vector.tensor_tensor(
                out=r[:, :, :].rearrange("s h (k c) -> s h k c", k=2),
                in0=src[:, :, :].rearrange("s h (k c) -> s h k c", k=2),
                in1=cos_b, op=ALU.mult)
            tmp = sb.tile([S, H, C], F32, tag="rope_t")
            # tmp_lo = q_hi * sin ; tmp_hi = q_lo * (-sin)
            nc.vector.tensor_tensor(
                out=tmp[:, :, 0:64], in0=src[:, :, 64:128],
                in1=sinD[:, 0:1, :].to_broadcast([S, H, 64]), op=ALU.mult)
            nc.vector.tensor_tensor(
                out=tmp[:, :, 64:128], in0=src[:, :, 0:64],
                in1=sinD[:, 1:2, :].to_broadcast([S, H, 64]), op=ALU.mult)
            nc.vector.tensor_add(out=r[:, :, :], in0=r[:, :, :],
                                 in1=tmp[:, :, :])
            sqq = sb.tile([S, H, C], F32, tag="rope_sq")
            nc.scalar.activation(out=sqq[:, :, :], in_=r[:, :, :],
                                 func=AF.Square)
            ssq = sb.tile([S, H], F32, tag="rope_ss")
            nc.vector.tensor_reduce(out=ssq[:, :], in_=sqq[:, :, :],
                                    axis=AX.X, op=ALU.add)
            sf = sb.tile([S, H], F32, tag="rope_sf")
            nc.scalar.activation(out=sf[:, :], in_=ssq[:, :], func=AF.Sqrt,
                                 bias=eps_t[0:S, 0:1], scale=1.0 / C)
            rf = sb.tile([S, H], F32, tag="rope_rf")
            nc.vector.reciprocal(out=rf[:, :], in_=sf[:, :])
            nc.vector.tensor_scalar_mul(rf[:, :], rf[:, :], tao_col)
            nc.vector.tensor_tensor(
                out=dst[:, :, :], in0=r[:, :, :],
                in1=rf[:, :].rearrange("s (h a) -> s h a", a=1)
                    .to_broadcast([S, H, C]), op=ALU.mult)

        qn = sb.tile([S, H, C], F32, tag="qn")
        kn = sb.tile([S, H, C], F32, tag="kn")
        rope_norm(q_all, qn, taob[:, 0:1])
        rope_norm(k_all, kn, taob[:, 1:2])

        if LEVEL == 4:
            nc.sync.dma_start(out=out[:, :], in_=qn[0:NSEL, 0, :])
            return
        # ---------------- attention ----------------
        qnT = sb.tile([128, H, S], F32, tag="qnT")
        knT = sb.tile([128, H, S], F32, tag="knT")
        for si, (srcT, dstT) in enumerate(((qn, qnT), (kn, knT))):
            for g in range(2):
                base = 512 * ((2 * si + g) % 2)
                for j in range(4):
                    nc.tensor.matmul(
                        out=PS[:, base + j * S:base + (j + 1) * S],
                        lhsT=srcT[:, 4 * g + j, :],
                        rhs=ident_t[0:S, 0:S], start=True, stop=True)
                nc.vector.tensor_copy(
                    out=dstT[:, 4 * g:4 * g + 4, :],
                    in_=PS[:, base:base + 4 * S].rearrange(
                        "p (a b) -> p a b", a=4))

        att_ps = PS[0:S, 2560:3584].rearrange("s (h c) -> s h c", h=H)
        for h in range(H):
            nc.tensor.matmul(out=att_ps[:, h, 0:S], lhsT=qnT[:, h, :],
                             rhs=knT[:, h, :], start=True, stop=True)
        t0 = sb.tile([S, H, S], F32, tag="t0")
        nc.vector.tensor_tensor(
            out=t0[:, :, :], in0=att_ps[:, :, 0:S],
            in1=cmask_t[:, :].rearrange("s (a t) -> s a t", a=1)
                .to_broadcast([S, H, S]), op=ALU.add)
        m = sb.tile([S, H], F32, tag="rowmax")
        nc.vector.tensor_reduce(out=m[:, :], in_=t0[:, :, :], axis=AX.X,
                                op=ALU.max)
        mneg = sb.tile([S, H], F32, tag="mneg")
        nc.vector.tensor_scalar_mul(mneg[:, :], m[:, :], -SCALE)
        p_sb = sb.tile([S, H, S], F32, tag="p_sb")
        den = sb.tile([S, H], F32, tag="den")
        for h in range(H):
            nc.scalar.activation(out=p_sb[:, h, :], in_=t0[:, h, :],
                                 func=AF.Exp, bias=mneg[:, h:h + 1],
                                 scale=SCALE, accum_out=den[:, h:h + 1])
        pT = sb.tile([S, H, S], F32, tag="pT")
        for g in range(2):
            base = 512 * (g % 2)
            for j in range(4):
                nc.tensor.matmul(
                    out=PS[0:S, base + j * S:base + (j + 1) * S],
                    lhsT=p_sb[:, 4 * g + j, :],
                    rhs=ident_t[0:S, 0:S], start=True, stop=True)
            nc.scalar.copy(
                out=pT[:, 4 * g:4 * g + 4, :],
                in_=PS[0:S, base:base + 4 * S].rearrange(
                    "p (a b) -> p a b", a=4))

        v_sb = sb.tile([S, H, C], F32, tag="v_sb")
        nc.scalar.copy(out=v_sb[:, :, :], in_=v_all[:, :, :])
        # absorb the DVE tick of the t0 read (WAR release of the att
        # region), then the late ACT tick of the pT copies; both write
        # the same column so WAW chains them in program order
        nc.tensor.matmul(out=PS[0:S, 2560:2561], lhsT=t0[:, 0, :],
                         rhs=ident_t[0:S, 0:1], start=True, stop=True)
        nc.tensor.matmul(out=PS[0:S, 2560:2561], lhsT=pT[:, 7, :],
                         rhs=ident_t[0:S, 0:1], start=True, stop=True)
        y_ps = PS[0:S, 2560:3584].rearrange("s (h c) -> s h c", h=H)
        for h in range(H):
            nc.tensor.matmul(out=y_ps[:, h, :], lhsT=pT[:, h, :],
                             rhs=v_sb[:, h, :], start=True, stop=True)

        rden = sb.tile([S, H], F32, tag="rden")
        nc.vector.reciprocal(out=rden[:, :], in_=den[:, :])
        sigg = sb.tile([NSEL, H, C], F32, tag="sigg")
        nc.scalar.activation(out=sigg[:, :, :], in_=g_all[:, :, :],
                             func=AF.Sigmoid)
        yg = sb.tile([NSEL, H, C], F32, tag="yg")
        nc.vector.tensor_tensor(
            out=yg[:, :, :], in0=y_ps[0:NSEL, :, :],
            in1=rden[0:NSEL, :].rearrange("s (h a) -> s h a", a=1)
                .to_broadcast([NSEL, H, C]), op=ALU.mult)
        nc.vector.tensor_tensor(out=yg[:, :, :], in0=yg[:, :, :],
                                in1=sigg[:, :, :], op=ALU.mult)

        if LEVEL == 5:
            nc.sync.dma_start(out=out[:, :], in_=yg[:, 0, :])
            return
        # ---------------- output projection ----------------
        ygT = sb.tile([128, H, NSEL], F32, tag="ygT")
        nc.vector.tensor_copy(out=touch[0:S, 14:15], in_=pT[:, 7, 0:1])
        for g in range(2):
            base = 512 * (g % 2)
            for j in range(4):
                nc.tensor.matmul(
                    out=PS[:, base + j * NSEL:base + (j + 1) * NSEL],
                    lhsT=yg[:, 4 * g + j, :],
                    rhs=ident_t[0:NSEL, 0:NSEL], start=True, stop=True)
            nc.vector.tensor_copy(
                out=ygT[:, 4 * g:4 * g + 4, :],
                in_=PS[:, base:base + 4 * NSEL].rearrange(
                    "p (a b) -> p a b", a=4))

        out_ps = PS[0:NSEL, 2048:2176]
        for h in range(H):
            nc.tensor.matmul(out=out_ps, lhsT=ygT[:, h, :],
                             rhs=woT[:, h, :], start=(h == 0),
                             stop=(h == H - 1))
        out_sb = sb.tile([NSEL, C], F32, tag="out_sb")
        nc.scalar.copy(out=out_sb[:, :], in_=out_ps)
        nc.sync.dma_start(out=out[:, :], in_=out_sb[:, :])


def make_host_constants():
    ident = np.eye(128, dtype=np.float32)
    off16 = (np.arange(NSEL, dtype=np.float32) % T0).reshape(NSEL, 1)
    negio = (float(NP) - np.arange(NP, dtype=np.float32)).reshape(1, NP)
    # partition p < 64 holds sequence position p+1; partition 64 is the
    # sink (position 0)
    pos = np.where(np.arange(S) < NSEL, np.arange(S) + 1, 0)
    cmask = np.where(pos[None, :] <= pos[:, None], 0.0,
                     NEG_BIG).astype(np.float32)
    m_idx = np.arange(NSEL)
    repmat = (16.0 * (m_idx[None, :] // 16 ==
                      np.arange(4)[:, None])).astype(np.float32)
    return ident, off16, negio, cmask, repmat


_CACHE = {}


def get_nc():
    if "nc" not in _CACHE:
        nc = bacc.Bacc("TRN2", target_bir_lowering=False, debug=False,
                       num_devices=B)
        build_kernel(nc)
        nc.compile()
        _CACHE["nc"] = nc
    return _CACHE["nc"]


def make_in_maps(inputs):
    x = np.ascontiguousarray(inputs["x"], dtype=np.float32)
    cos = np.ascontiguousarray(np.asarray(inputs["cos"]).reshape(S, 64),
                               dtype=np.float32)
    sin = np.ascontiguousarray(np.asarray(inputs["sin"]).reshape(S, 64),
                               dtype=np.float32)
    sinkv = np.ascontiguousarray(np.asarray(inputs["sink"]).reshape(H, C),
                                 dtype=np.float32)
    wqkvg = np.ascontiguousarray(inputs["W_qkvg"], dtype=np.float32)
    pw = np.ascontiguousarray(inputs["patch_w"], dtype=np.float32)
    wout = np.ascontiguousarray(inputs["W_out"], dtype=np.float32)
    tao = np.ascontiguousarray(
        np.broadcast_to(np.asarray(inputs["tao"], dtype=np.float32), (S, 2)))
    ident, off16, negio, cmask, repmat = make_host_constants()
    in_maps = []
    for b in range(B):
        in_maps.append({
            "xb": np.ascontiguousarray(x[b]),
            "pw": pw, "wqkvg": wqkvg, "wout": wout, "sink": sinkv,
            "cosd": cos, "sind": sin, "tao": tao, "ident": ident,
            "off16": off16, "negio": negio, "cmask": cmask,
            "repmat": repmat,
        })
    return in_maps


def kernel(**inputs):
    nc = get_nc()
    in_maps = make_in_maps(inputs)
    res = run_bass_kernel_spmd(nc, in_maps, core_ids=list(range(B)))
    return np.stack([r["out"] for r in res.results], axis=0)


if __name__ == "__main__":
    nc = get_nc()
    print("build ok:", len(nc.m.functions[0].allocations), "allocations")



# revision 32
# speedup vs baseline: 2.0488x; 2.0488x over previous
"""Trainium2 Bass kernel for nn_AttentionOnDetail (sparse patch attention).

Data-parallel over batch B=8 across 8 NeuronCores; one batch per core.

v2 design (latency-focused; the kernel is dependency-bound, not
throughput-bound):
  - Host-side prep inside kernel(): W_qkvg.T / W_out.T passed
    pre-transposed, cos/sin tables pre-permuted and duplicated,
    rmsnorm(sink)*tao rows precomputed (rope at position 0 is identity).
  - x tile DMAs issued first; patch stats (ACT square+accum, DVE dot)
    pipeline behind them; per-tile logits transposed into a PSUM row via
    PE so top-4 selection needs no DMA.
  - Top-4 via max8/threshold/max_index (ascending patch order); token
    gather via one indirect DMA of 4 whole patches.
  - qkvg projection computed "PE-direct": per (tensor, kilo-block)
    matmuls with column-selected lhsT views and stride-4 PSUM output
    rows land q/k/v/g directly in attention layout (no DRAM bounce, no
    rearrange DMAs).  fp32r operands -> 1 cycle/row.
  - q and k stacked on 128 partitions: rmsnorm+rope for both costs one
    set of full-width DVE ops (cost scales with free size only).
    rsqrt via ln+exp keeps ACT on a single function table; tao folded
    into the exp bias.
  - attention: bf16 matmuls, no row-max softmax (range is bounded),
    one exp over all heads, 1/den folded into p before the transpose.
"""

import sys
import numpy as np

for _p in ("/opt/trn_rl_repo",):
    if _p not in sys.path:
        sys.path.insert(0, _p)

import concourse.bass as bass
import concourse.bacc as bacc
import concourse.tile as tile
from concourse import mybir
from concourse.bass_utils import run_bass_kernel_spmd

F32 = mybir.dt.float32
F32R = mybir.dt.float32r
BF16 = mybir.dt.bfloat16
I32 = mybir.dt.int32
U32 = mybir.dt.uint32
U16 = mybir.dt.uint16
AF = mybir.ActivationFunctionType
ALU = mybir.AluOpType
AX = mybir.AxisListType

B, T, C, H, T0 = 8, 8192, 128, 8, 16
NP = T // T0          # 512 patches
PATCH = T0 * C        # 2048 elements per patch
S = 65                # sink + 64 selected tokens
NSEL = 64
EPS = 1.1920929e-07
SCALE = 1.0 / float(np.sqrt(np.float32(C)))
NEG_BIG = -1.0e30


def rap(t, apl, offset=0):
    """Raw AP over a tile/AP's storage, flat element strides."""
    base = t if isinstance(t, bass.AP) else t[:]
    return bass.AP(tensor=base.tensor, offset=base.offset + offset,
                   ap=[list(x) for x in apl])


def f32r(ap):
    return ap.bitcast(F32R)


def build_kernel(nc):
    xb = nc.dram_tensor("xb", [T, C], F32, kind="ExternalInput")
    pw = nc.dram_tensor("pw", [PATCH], F32R, kind="ExternalInput")
    wqT_d = nc.dram_tensor("wqT_d", [C, 4 * C * H], F32R, kind="ExternalInput")
    woT_d = nc.dram_tensor("woT_d", [C, H, C], F32, kind="ExternalInput")
    cosdup = nc.dram_tensor("cosdup", [128, C], F32, kind="ExternalInput")
    sinpm = nc.dram_tensor("sinpm", [128, C], F32, kind="ExternalInput")
    cmask = nc.dram_tensor("cmask", [S, S], F32, kind="ExternalInput")
    negio = nc.dram_tensor("negio", [1, NP], F32, kind="ExternalInput")
    sinkTq = nc.dram_tensor("sinkTq", [C, H], F32, kind="ExternalInput")
    sinkTk = nc.dram_tensor("sinkTk", [C, H], F32, kind="ExternalInput")
    sinkvb = nc.dram_tensor("sinkvb", [1, H * C], U16, kind="ExternalInput")
    sel16 = nc.dram_tensor("sel16", [5, NSEL], F32, kind="ExternalInput")
    onesd = nc.dram_tensor("onesd", [1, 128], F32R, kind="ExternalInput")
    ident = nc.dram_tensor("ident", [128, 128], F32, kind="ExternalInput")
    out = nc.dram_tensor("out", [NSEL, C], F32, kind="ExternalOutput")

    with tile.TileContext(nc) as tc:
        _emit(tc, nc, xb, pw, wqT_d, woT_d, cosdup, sinpm, cmask,
              negio, sinkTq, sinkTk, sinkvb, sel16, onesd, ident, out)
    return nc


def _emit(tc, nc, xb, pw, wqT_d, woT_d, cosdup, sinpm, cmask,
          negio, sinkTq, sinkTk, sinkvb, sel16, onesd, ident, out):
    import os
    LEVEL = int(os.environ.get("KLEVEL", "9"))
    from contextlib import ExitStack
    ctx = ExitStack()
    with ctx:
        const1 = ctx.enter_context(tc.tile_pool(name="const1", bufs=1))
        xpool = ctx.enter_context(tc.tile_pool(name="xpool", bufs=1))
        junkp = ctx.enter_context(tc.tile_pool(name="junkp", bufs=1))
        stat = ctx.enter_context(tc.tile_pool(name="stat", bufs=4))
        sb = ctx.enter_context(tc.tile_pool(name="sb", bufs=1))
        psall = ctx.enter_context(tc.tile_pool(name="psall", bufs=1,
                                               space="PSUM"))
        # one tile owning all 8 PSUM banks; regions choreographed manually
        PS = psall.tile([128, 4096], F32)
        # region plan (f32 cols):
        #   0:1024     qk stack (q rows 0:64, k rows 64:128); later att
        #              [65, 520] at 0:520; later out [64, 128]
        #   1024:2048  v rows 0:64; later pT staging [65, 260] at 1024:1284
        #   2048:3072  g rows 0:64
        #   2560:3584  y [65, 1024] (after sigg consumed g)
        #   3072:3584  staging A (pw bcast, x_selT, qnT)
        #   3584:4096  logits row (rows 0:1) / knT staging / ygT staging
        LROW = 3584

        # ---------------- x tile 0 first, then tiny tables ----------------
        def xdma(i):
            xp = xpool.tile([128, PATCH], F32, tag=f"xp{i}")
            nc.sync.dma_start(
                out=xp[:, :],
                in_=rap(xb[:, :], [[PATCH, 128], [1, PATCH]],
                        offset=i * 128 * PATCH))
            return xp

        xps = [xdma(0)]
        ident_t = const1.tile([128, 128], F32)
        nc.sync.dma_start(out=ident_t[:, :], in_=ident[:, :])

        pw_sb = const1.tile([1, PATCH], F32R)
        nc.sync.dma_start(out=pw_sb[:, :], in_=rap(pw[:], [[1, 1], [1, PATCH]]))
        ones_t = const1.tile([1, 128], F32R)
        nc.sync.dma_start(out=ones_t[:, :], in_=onesd[:, :])
        eps_t = const1.tile([128, 1], F32)
        nc.vector.memset(eps_t[:, :], EPS)

        # ---------------- remaining x tiles ----------------
        for i in range(1, 4):
            xps.append(xdma(i))
        cosdup_t = const1.tile([128, C], F32)
        nc.sync.dma_start(out=cosdup_t[:, :], in_=cosdup[:, :])
        sinpm_t = const1.tile([128, C], F32)
        nc.sync.dma_start(out=sinpm_t[:, :], in_=sinpm[:, :])
        cmask_t = const1.tile([S, S], F32)
        nc.sync.dma_start(out=cmask_t[:, :], in_=cmask[:, :])
        negio_t = const1.tile([1, NP], F32)
        nc.sync.dma_start(out=negio_t[:, :], in_=negio[:, :])
        sinkTq_t = const1.tile([C, H], F32)
        nc.sync.dma_start(out=sinkTq_t[:, :], in_=sinkTq[:, :])
        sinkTk_t = const1.tile([C, H], F32)
        nc.sync.dma_start(out=sinkTk_t[:, :], in_=sinkTk[:, :])
        sel16_t = const1.tile([5, NSEL], F32)
        nc.sync.dma_start(out=sel16_t[:, :], in_=sel16[:, :])
        rhs5 = const1.tile([5, 1], F32)
        nc.vector.memset(rhs5[:, :], 1.0)

        # v sink row: host-rounded bf16 bits straight into v_sb row 64
        v_sb = sb.tile([S, H, C], BF16, tag="v_sb")
        nc.sync.dma_start(
            out=v_sb[NSEL:S, :, :],
            in_=sinkvb[:, :].bitcast(BF16).rearrange(
                "p (h c) -> p h c", h=H))

        wqT = const1.tile([C, 4 * C * H], F32R)
        for wch in range(4):
            nc.sync.dma_start(out=wqT[:, 1024 * wch:1024 * (wch + 1)],
                              in_=wqT_d[:, 1024 * wch:1024 * (wch + 1)])
        woT = const1.tile([C, H, C], F32)


        # preload the sqrt activation table while ACT is idle
        dummy = stat.tile([1, 1], F32)
        nc.vector.memset(dummy[:, :], 1.0)
        nc.scalar.activation(out=dummy[:, :], in_=dummy[:, :],
                             func=AF.Sqrt)

        woTb = const1.tile([C, H, C], BF16)

        # pw broadcast to 128 partitions via K=1 matmul into staging banks
        # (two tiny warmups first lift PE off the cold p-state)
        nc.tensor.matmul(out=PS[0:128, 3071:3072], lhsT=ident_t[:, :],
                         rhs=ident_t[:, 0:1], start=True, stop=True)
        nc.tensor.matmul(out=PS[0:128, 3071:3072], lhsT=ident_t[:, :],
                         rhs=ident_t[:, 0:1], start=True, stop=True)
        pwB = const1.tile([128, PATCH], F32)
        for q4 in range(4):
            base = 3072 + 512 * (q4 % 2)
            nc.tensor.matmul(out=PS[:, base:base + 512],
                             lhsT=ones_t[:, :],
                             rhs=pw_sb[:, 512 * q4:512 * (q4 + 1)],
                             start=True, stop=True)
            if q4 % 2 == 0:
                nc.scalar.copy(out=pwB[:, 512 * q4:512 * (q4 + 1)],
                               in_=PS[:, base:base + 512])
            else:
                nc.vector.tensor_copy(out=pwB[:, 512 * q4:512 * (q4 + 1)],
                                      in_=PS[:, base:base + 512])

        # ---------------- phase 1: per-patch stats ----------------
        junk = junkp.tile([128, PATCH], F32, tag="junk")
        junk2 = junkp.tile([128, PATCH], F32, tag="junk2")
        ss_c = stat.tile([128, 4], F32, tag="ss_c")
        dot_c = stat.tile([128, 4], F32, tag="dot_c")
        rs_c = stat.tile([128, 4], F32, tag="rs_c")
        logit_c = stat.tile([128, 4], F32, tag="logit_c")
        for i in range(4):
            xp = xps[i]
            nc.scalar.activation(out=junk[:, :], in_=xp[:, :], func=AF.Square,
                                 accum_out=ss_c[:, i:i + 1])
            nc.vector.scalar_tensor_tensor(
                out=junk2[:, :], in0=xp[:, :], scalar=1.0, in1=pwB[:, :],
                op0=ALU.mult, op1=ALU.mult, accum_out=dot_c[:, i:i + 1])
            nc.scalar.activation(out=rs_c[:, i:i + 1], in_=ss_c[:, i:i + 1],
                                 func=AF.Sqrt, bias=eps_t[:, :],
                                 scale=1.0 / PATCH)
            nc.vector.reciprocal(out=rs_c[:, i:i + 1], in_=rs_c[:, i:i + 1])
            nc.vector.tensor_mul(logit_c[:, i:i + 1], dot_c[:, i:i + 1],
                                 rs_c[:, i:i + 1])
            # transpose this tile's logits column into the PSUM row
            nc.tensor.transpose(
                out=PS[0:1, LROW + 128 * i:LROW + 128 * (i + 1)],
                in_=logit_c[:, i:i + 1], identity=ident_t[:, :])

        if LEVEL == 1:
            lrow_sb = stat.tile([1, NP], F32, tag="lrow_sb")
            nc.vector.tensor_copy(out=lrow_sb[:, :],
                                  in_=PS[0:1, LROW:LROW + NP])
            for r in range(4):
                nc.sync.dma_start(out=out[r:r + 1, :],
                                  in_=lrow_sb[0:1, 128 * r:128 * (r + 1)])
            return

        # ---------------- top-4 selection (on the PSUM row) ----------------
        lrow = PS[0:1, LROW:LROW + NP]
        max8 = stat.tile([1, 8], F32, tag="max8")
        nc.vector.max(out=max8[:, :], in_=lrow)
        masked = stat.tile([1, NP], F32, tag="masked")
        nc.vector.scalar_tensor_tensor(
            out=masked[:, :], in0=lrow, scalar=max8[:, 3:4],
            in1=negio_t[:, :], op0=ALU.is_ge, op1=ALU.mult)
        mm8 = stat.tile([1, 8], F32, tag="mm8")
        nc.vector.max(out=mm8[:, :], in_=masked[:, :])
        idx8 = stat.tile([1, 8], U32, tag="idx8")
        nc.vector.max_index(out=idx8[:, :], in_max=mm8[:, :],
                            in_values=masked[:, :])
        idxf = stat.tile([1, 8], F32, tag="idxf")
        nc.vector.tensor_copy(out=idxf[:, :], in_=idx8[:, :])

        # patch-id column via PE transpose: [1,4] -> [4,1], then token
        # ids 16*I[p] + 4T + t in (T, p, t) row order via sel16
        nc.tensor.transpose(out=PS[0:4, 3582:3583], in_=idxf[0:1, 0:4],
                            identity=ident_t[0:1, 0:1])
        nc.scalar.copy(out=rhs5[0:4, :], in_=PS[0:4, 3582:3583])
        nc.tensor.matmul(out=PS[0:NSEL, 3583:3584], lhsT=sel16_t[:, :],
                         rhs=rhs5[:, :], start=True, stop=True)
        idc_f = stat.tile([NSEL, 1], F32, tag="idc_f")
        nc.scalar.copy(out=idc_f[:, :], in_=PS[0:NSEL, 3583:3584])
        idc_i = stat.tile([NSEL, 1], I32, tag="idc_i")
        nc.vector.tensor_copy(out=idc_i[:, :], in_=idc_f[:, :])

        # gather the 64 tokens (row 16T+4p+t = token 16*I[p] + 4T + t)
        x_sel = sb.tile([NSEL, C], F32, tag="x_sel")
        nc.gpsimd.indirect_dma_start(
            out=x_sel[:, :], out_offset=None, in_=xb[:, :],
            in_offset=bass.IndirectOffsetOnAxis(ap=idc_i[:, 0:1], axis=0))

        if LEVEL == 2:
            nc.sync.dma_start(out=out[:, :], in_=x_sel[:, :])
            return

        # ---------------- qkvg projection ----------------
        nc.tensor.transpose(out=PS[0:128, 3072:3072 + NSEL], in_=x_sel[:, :],
                            identity=ident_t[0:NSEL, 0:NSEL])
        x_selT = sb.tile([C, NSEL], F32R, tag="x_selT")
        nc.scalar.copy(out=x_selT[:, :], in_=PS[:, 3072:3072 + NSEL])

        # qkvg[token, f] for the 64 gathered tokens -> PS rows 0:64
        for g in range(8):
            nc.tensor.matmul(out=PS[0:NSEL, 512 * g:512 * (g + 1)],
                             lhsT=x_selT[:, :],
                             rhs=wqT[:, 512 * g:512 * (g + 1)],
                             start=True, stop=True)
        qkvg_sb = sb.tile([NSEL, 4 * C * H], BF16, tag="qkvg_sb")
        nc.scalar.copy(out=qkvg_sb[:, 0:1024], in_=PS[0:NSEL, 0:1024])
        nc.vector.tensor_copy(out=qkvg_sb[:, 1024:2048],
                              in_=PS[0:NSEL, 1024:2048])
        nc.scalar.copy(out=qkvg_sb[:, 2048:3072], in_=PS[0:NSEL, 2048:3072])
        nc.vector.tensor_copy(out=qkvg_sb[:, 3072:4096],
                              in_=PS[0:NSEL, 3072:4096])

        # rearrange token-major -> s-major via SBUF->SBUF DMAs.
        # qkvg row 16T+4p+t (token 16*I[p]+4T+t), col (b,h,c) feeds
        # s-row 16p+4t+b of tensor T: per tensor the source rows are the
        # contiguous block 16T:16T+16 -> clean single-stride APs.
        qk = sb.tile([128, H, C], BF16, tag="qk")
        vg = sb.tile([128, H, C], BF16, tag="vg")
        FQ = 4 * C * H

        def rearr(tens, dst, half, eng):
            eng.dma_start(
                out=dst[64 * half:64 * half + NSEL, :, :],
                in_=rap(qkvg_sb[:, :], [[FQ, T0], [1024, 4], [1, 1024]],
                        offset=T0 * tens * FQ))

        rearr(0, qk, 0, nc.sync)    # q on the SP queue
        rearr(1, qk, 1, nc.scalar)  # k on the ACT queue (packs transfers)

        # out-projection weights arrive late; the dummy write makes the DMA
        # wait for the gather so it cannot block the gather's transfer
        nc.vector.tensor_copy(out=woT[0:1, 0, 0:1], in_=x_sel[0:1, 0:1])
        nc.sync.dma_start(out=woT[:, :, :], in_=woT_d[:, :, :])
        nc.gpsimd.tensor_copy(out=woTb[:, :, :], in_=woT[:, :, :])

        if LEVEL == 3:
            q0 = sb.tile([NSEL, C], F32, tag="q0dbg")
            nc.vector.tensor_copy(out=q0[:, :], in_=qk[0:NSEL, 0, :])
            nc.sync.dma_start(out=out[:, :], in_=q0[:, :])
            return

        # ---------------- rmsnorm + rope on the qk stack ----------------
        ssq = sb.tile([128, H], F32, tag="ssq")
        sqj = junkp.tile([128, H, C], F32, tag="sqj")
        nc.gpsimd.tensor_tensor(out=sqj[:, 5:8, :], in0=qk[:, 5:8, :],
                                in1=qk[:, 5:8, :], op=ALU.mult)
        nc.vector.tensor_tensor(out=sqj[:, 0:5, :], in0=qk[:, 0:5, :],
                                in1=qk[:, 0:5, :], op=ALU.mult)
        nc.vector.tensor_reduce(out=ssq[:, 0:5], in_=sqj[:, 0:5, :],
                                axis=AX.X, op=ALU.add)
        nc.vector.tensor_reduce(out=ssq[:, 5:8], in_=sqj[:, 5:8, :],
                                axis=AX.X, op=ALU.add)
        # Release the v/g rearranges only after the reduce so sigmoid's ACT
        # table load cannot be scheduled ahead of the rope sqrt: a genuine
        # RAW chain -- zro = 0*ssq, then corner += zro on one element of
        # each of the v/g source row blocks (value-preserving).
        zro = stat.tile([17, 1], F32, tag="zro")
        nc.vector.tensor_scalar_mul(zro[:, :], ssq[0:17, 0:1], 0.0)
        corner = rap(qkvg_sb[:, :], [[FQ, 17], [1, 1]], offset=32 * FQ)
        nc.vector.tensor_scalar(out=corner, in0=corner,
                                scalar1=zro[:, 0:1], scalar2=None,
                                op0=ALU.add)
        rearr(2, vg, 0, nc.sync)
        rearr(3, vg, 1, nc.sync)
        rf = sb.tile([128, H], F32, tag="rf")
        nc.scalar.activation(out=rf[:, :], in_=ssq[:, :], func=AF.Sqrt,
                             bias=eps_t[:, :], scale=1.0 / C)
        nc.vector.reciprocal(out=rf[:, :], in_=rf[:, :])
        # v -> bf16 for the y matmul; sigmoid gate from the g half (the
        # table load + sigg hide under the DVE rope chain; exp reloads
        # ln/exp before the softmax)
        nc.scalar.copy(out=v_sb[0:NSEL, :, :], in_=vg[0:NSEL, :, :])
        sigg = sb.tile([NSEL, H, C], BF16, tag="sigg")
        nc.scalar.activation(out=sigg[:, :, :], in_=vg[NSEL:128, :, :],
                             func=AF.Sigmoid)
        qk1 = sb.tile([128, H, C], F32, tag="qk1")
        r1 = sb.tile([128, H, C], F32, tag="r1")
        r2 = sb.tile([128, H, C], F32, tag="r2")
        qkn = sb.tile([128, H, C], F32, tag="qkn")

        def hs(eng, hs0, hs1):
            hn = hs1 - hs0
            eng.tensor_tensor(
                out=qk1[:, hs0:hs1, :], in0=qk[:, hs0:hs1, :],
                in1=rf[:, hs0:hs1].rearrange("p (h a) -> p h a", a=1)
                    .to_broadcast([128, hn, C]), op=ALU.mult)
            eng.tensor_tensor(
                out=r1[:, hs0:hs1, :], in0=qk1[:, hs0:hs1, :],
                in1=cosdup_t[:, :].rearrange("p (a c) -> p a c", a=1)
                    .to_broadcast([128, hn, C]), op=ALU.mult)
            eng.tensor_tensor(
                out=r2[:, hs0:hs1, 0:64], in0=qk1[:, hs0:hs1, 64:128],
                in1=sinpm_t[:, 0:64].rearrange("p (a c) -> p a c", a=1)
                    .to_broadcast([128, hn, 64]), op=ALU.mult)
            eng.tensor_tensor(
                out=r2[:, hs0:hs1, 64:128], in0=qk1[:, hs0:hs1, 0:64],
                in1=sinpm_t[:, 64:128].rearrange("p (a c) -> p a c", a=1)
                    .to_broadcast([128, hn, 64]), op=ALU.mult)
            eng.tensor_add(out=qkn[:, hs0:hs1, :], in0=r1[:, hs0:hs1, :],
                           in1=r2[:, hs0:hs1, :])

        hs(nc.vector, 0, 5)
        hs(nc.gpsimd, 5, 8)

        if LEVEL == 4:
            qn32 = sb.tile([NSEL, C], F32, tag="qn32")
            nc.vector.tensor_copy(out=qn32[:, :], in_=qkn[0:NSEL, 0, :])
            nc.sync.dma_start(out=out[:, :], in_=qn32[:, :])
            return

        # ---------------- transposes to qnT / knT ----------------
        qnT = sb.tile([C, H, S], BF16, tag="qnT")
        knT = sb.tile([C, H, S], BF16, tag="knT")
        for si, dstT in enumerate((qnT, knT)):
            base = 3072 + 512 * si
            for h in range(H):
                nc.tensor.transpose(
                    out=PS[:, base + NSEL * h:base + NSEL * (h + 1)],
                    in_=qkn[64 * si:64 * (si + 1), h, :],
                    identity=ident_t[64 * si:64 * si + NSEL,
                                     64 * si:64 * si + NSEL])
            cpeng = nc.vector if si == 0 else nc.scalar
            if si == 0:
                nc.vector.tensor_copy(
                    out=rap(dstT[:, :, :], [[H * S, C], [S, H], [1, NSEL]]),
                    in_=PS[:, base:base + 512].rearrange(
                        "p (h s) -> p h s", h=H))
            else:
                nc.scalar.copy(
                    out=rap(dstT[:, :, :], [[H * S, C], [S, H], [1, NSEL]]),
                    in_=PS[:, base:base + 512].rearrange(
                        "p (h s) -> p h s", h=H))
        # sink columns (position 0: rope is identity; host prenormed)
        nc.scalar.copy(out=rap(qnT[:, :, :], [[H * S, C], [S, H], [1, 1]],
                               offset=NSEL),
                       in_=sinkTq_t[:, :].rearrange("c (h a) -> c h a", a=1))
        nc.scalar.copy(out=rap(knT[:, :, :], [[H * S, C], [S, H], [1, 1]],
                               offset=NSEL),
                       in_=sinkTk_t[:, :].rearrange("c (h a) -> c h a", a=1))

        # ---------------- attention ----------------
        # att head slots padded to 128 cols (matmul must not cross banks)
        att_ps = rap(PS[:, :], [[4096, S], [C, H], [1, S]])
        for h in range(H):
            nc.tensor.matmul(out=PS[0:S, C * h:C * h + S], lhsT=qnT[:, h, :],
                             rhs=knT[:, h, :], start=True, stop=True)
        t0 = sb.tile([S, H, S], F32, tag="t0")
        attv = rap(PS[:, :], [[4096, S], [C, H], [1, S]])
        nc.vector.tensor_tensor(
            out=t0[:, :, :], in0=attv,
            in1=cmask_t[:, :].rearrange("s (a t) -> s a t", a=1)
                .to_broadcast([S, H, S]), op=ALU.add)
        p_sb = sb.tile([S, H, S], F32, tag="p_sb")
        nc.scalar.activation(out=p_sb[:, :, :], in_=t0[:, :, :],
                             func=AF.Exp, scale=SCALE)
        den8 = sb.tile([S, H], F32, tag="den8")
        nc.vector.tensor_reduce(out=den8[:, :], in_=p_sb[:, :, :],
                                axis=AX.X, op=ALU.add)
        rden = sb.tile([S, H], F32, tag="rden")
        nc.vector.reciprocal(out=rden[:, :], in_=den8[:, :])
        # 1/den folds into the gate (runs in the pT/y shadow): the pT/y
        # matmuls consume UNNORMALIZED p; yg = y_raw * (sigg * rden)
        sigrd = sb.tile([NSEL, H, C], BF16, tag="sigrd")
        nc.vector.tensor_tensor(
            out=sigrd[:, :, :], in0=sigg[:, :, :],
            in1=rden[0:NSEL, :].rearrange("s (h a) -> s h a", a=1)
                .to_broadcast([NSEL, H, C]), op=ALU.mult)

        # pT: transpose p per head -> [t, h, s]
        pT = sb.tile([S, H, S], BF16, tag="pT")
        for g in range(2):
            for j in range(4):
                nc.tensor.transpose(
                    out=PS[0:S, 1024 + S * j:1024 + S * (j + 1)],
                    in_=p_sb[:, 4 * g + j, :], identity=ident_t[0:S, 0:S])
            nc.vector.tensor_copy(
                out=pT[:, 4 * g:4 * g + 4, :],
                in_=PS[0:S, 1024:1024 + 4 * S].rearrange(
                    "p (a b) -> p a b", a=4))

        # y = pT^T @ v per head -> PS [65, 1024] at cols 2560:3584
        y_ps = PS[0:S, 2560:3584].rearrange("s (h c) -> s h c", h=H)
        for h in range(H):
            nc.tensor.matmul(out=y_ps[:, h, :], lhsT=pT[:, h, :],
                             rhs=v_sb[:, h, :], start=True, stop=True)
        yg = sb.tile([NSEL, H, C], F32, tag="yg")
        nc.vector.tensor_tensor(out=yg[:, :, :], in0=y_ps[0:NSEL, :, :],
                                in1=sigrd[:, :, :], op=ALU.mult)

        if LEVEL == 5:
            yg32 = sb.tile([NSEL, C], F32, tag="yg32")
            nc.vector.tensor_copy(out=yg32[:, :], in_=yg[0:NSEL, 0, :])
            nc.sync.dma_start(out=out[:, :], in_=yg32[:, :])
            return

        # ---------------- output projection ----------------
        ygT = sb.tile([C, H, NSEL], BF16, tag="ygT")
        for h in range(H):
            nc.tensor.transpose(
                out=PS[:, LROW + NSEL * h:LROW + NSEL * (h + 1)],
                in_=yg[:, h, :], identity=ident_t[0:NSEL, 0:NSEL])
        nc.vector.tensor_copy(
            out=ygT[:, :, :],
            in_=PS[:, LROW:LROW + 512].rearrange("p (h s) -> p h s", h=H))

        out_sb = sb.tile([NSEL, C], F32, tag="out_sb")
        for half in range(2):
            cols = slice(64 * half, 64 * (half + 1))
            out_ps = PS[0:NSEL, 64 * half:64 * (half + 1)]
            for h in range(H):
                nc.tensor.matmul(out=out_ps, lhsT=ygT[:, h, :],
                                 rhs=woTb[:, h, cols], start=(h == 0),
                                 stop=(h == H - 1))
            nc.scalar.copy(out=out_sb[:, cols], in_=out_ps)
            nc.sync.dma_start(out=out[:, cols], in_=out_sb[:, cols])


def make_host_constants(inputs):
    """Host-side prep of tables derived from the (full) inputs."""
    cos = np.asarray(inputs["cos"]).reshape(S, 64).astype(np.float32)
    sin = np.asarray(inputs["sin"]).reshape(S, 64).astype(np.float32)
    sink = np.asarray(inputs["sink"]).reshape(H, C).astype(np.float32)
    tao = np.asarray(inputs["tao"]).astype(np.float32)
    wq = np.asarray(inputs["W_qkvg"]).astype(np.float32)
    wo = np.asarray(inputs["W_out"]).astype(np.float32)

    # partition p (0..63 in each half) holds position p+1; rows duplicated
    # for the q half (0:64) and k half (64:128)
    pos = np.arange(64) + 1
    cos_p = cos[pos]
    sin_p = sin[pos]
    cosdup = np.tile(np.concatenate([cos_p, cos_p], axis=1), (2, 1))
    sinpm = np.tile(np.concatenate([sin_p, -sin_p], axis=1), (2, 1))
    # tao folds into the rope tables: qn = (qk*rf)*cos' + swap(qk*rf)*sin'
    taocol = np.concatenate([np.full((64, 1), tao[0], np.float32),
                             np.full((64, 1), tao[1], np.float32)])
    cosdup = cosdup * taocol
    sinpm = sinpm * taocol

    # additive causal mask in s-major layout (row/col 64 = sink, pos 0)
    posf = np.where(np.arange(S) < NSEL, np.arange(S) + 1, 0)
    cmaskm = np.where(posf[None, :] <= posf[:, None], 0.0,
                      NEG_BIG).astype(np.float32)
    negio = (float(NP) - np.arange(NP, dtype=np.float32)).reshape(1, NP)

    # sink rows: rope at position 0 is identity; rmsnorm + tao on host
    sn = sink / np.sqrt((sink * sink).mean(axis=-1, keepdims=True) + EPS)
    sinkTq = np.ascontiguousarray((sn * tao[0]).T)
    sinkTk = np.ascontiguousarray((sn * tao[1]).T)
    # v sink row as bf16 bit pattern (round-to-nearest-even)
    f = sink.reshape(1, H * C).astype(np.float32)
    u = f.view(np.uint32)
    rounded = ((u + 0x7FFF + ((u >> 16) & 1)) >> 16).astype(np.uint16)
    sinkvb = np.ascontiguousarray(rounded)

    # token ids: row 16T+4p+t gathers token 16*I[p] + 4T + t
    # sel16[j, r] = 16*(j==p(r)) for j<4; sel16[4, r] = 4T(r) + t(r)
    sel16m = np.zeros((5, NSEL), np.float32)
    for Tn in range(4):
        for p in range(4):
            for t in range(4):
                r = 16 * Tn + 4 * p + t
                sel16m[p, r] = 16.0
                sel16m[4, r] = float(4 * Tn + t)

    wqT = np.ascontiguousarray(wq.T)
    woT = np.ascontiguousarray(wo.reshape(C, H, C).transpose(2, 1, 0))

    ident = np.eye(128, dtype=np.float32)
    return dict(cosdup=np.ascontiguousarray(cosdup, dtype=np.float32),
                sinpm=np.ascontiguousarray(sinpm, dtype=np.float32),
                cmask=cmaskm, negio=negio,
                sinkTq=sinkTq, sinkTk=sinkTk, sinkvb=sinkvb, sel16=sel16m,
                onesd=np.ones((1, 128), np.float32),
                wqT_d=wqT, woT_d=woT, ident=ident)


_CACHE = {}


def get_nc():
    if "nc" not in _CACHE:
        nc = bacc.Bacc("TRN2", target_bir_lowering=False, debug=False,
                       num_devices=B)
        build_kernel(nc)
        nc.compile()
        _CACHE["nc"] = nc
    return _CACHE["nc"]


def make_in_maps(inputs):
    x = np.ascontiguousarray(inputs["x"], dtype=np.float32)
    pwv = np.ascontiguousarray(inputs["patch_w"], dtype=np.float32)
    consts = make_host_constants(inputs)
    in_maps = []
    for b in range(B):
        m = {"xb": np.ascontiguousarray(x[b]), "pw": pwv}
        m.update(consts)
        in_maps.append(m)
    return in_maps


def kernel(**inputs):
    nc = get_nc()
    in_maps = make_in_maps(inputs)
    res = run_bass_kernel_spmd(nc, in_maps, core_ids=list(range(B)))
    return np.stack([r["out"] for r in res.results], axis=0)


if __name__ == "__main__":
    nc = get_nc()
    print("build ok:", len(nc.m.functions[0].allocations), "allocations")


# revision 35
# speedup vs baseline: 2.1012x; 1.0256x over previous
"""Trainium2 Bass kernel for nn_AttentionOnDetail (sparse patch attention).

Data-parallel over batch B=8 across 8 NeuronCores; one batch per core.

v2 design (latency-focused; the kernel is dependency-bound, not
throughput-bound):
  - Host-side prep inside kernel(): W_qkvg.T / W_out.T passed
    pre-transposed, cos/sin tables pre-permuted and duplicated,
    rmsnorm(sink)*tao rows precomputed (rope at position 0 is identity).
  - x tile DMAs issued first; patch stats (ACT square+accum, DVE dot)
    pipeline behind them; per-tile logits transposed into a PSUM row via
    PE so top-4 selection needs no DMA.
  - Top-4 via max8/threshold/max_index (ascending patch order); token
    gather via one indirect DMA of 4 whole patches.
  - qkvg projection computed "PE-direct": per (tensor, kilo-block)
    matmuls with column-selected lhsT views and stride-4 PSUM output
    rows land q/k/v/g directly in attention layout (no DRAM bounce, no
    rearrange DMAs).  fp32r operands -> 1 cycle/row.
  - q and k stacked on 128 partitions: rmsnorm+rope for both costs one
    set of full-width DVE ops (cost scales with free size only).
    rsqrt via ln+exp keeps ACT on a single function table; tao folded
    into the exp bias.
  - attention: bf16 matmuls, no row-max softmax (range is bounded),
    one exp over all heads, 1/den folded into p before the transpose.
"""

import sys
import numpy as np

for _p in ("/opt/trn_rl_repo",):
    if _p not in sys.path:
        sys.path.insert(0, _p)

import concourse.bass as bass
import concourse.bacc as bacc
import concourse.tile as tile
from concourse import mybir
from concourse.bass_utils import run_bass_kernel_spmd

F32 = mybir.dt.float32
F32R = mybir.dt.float32r
BF16 = mybir.dt.bfloat16
I32 = mybir.dt.int32
U32 = mybir.dt.uint32
U16 = mybir.dt.uint16
AF = mybir.ActivationFunctionType
ALU = mybir.AluOpType
AX = mybir.AxisListType

B, T, C, H, T0 = 8, 8192, 128, 8, 16
NP = T // T0          # 512 patches
PATCH = T0 * C        # 2048 elements per patch
S = 65                # sink + 64 selected tokens
NSEL = 64
EPS = 1.1920929e-07
SCALE = 1.0 / float(np.sqrt(np.float32(C)))
NEG_BIG = -1.0e30


def rap(t, apl, offset=0):
    """Raw AP over a tile/AP's storage, flat element strides."""
    base = t if isinstance(t, bass.AP) else t[:]
    return bass.AP(tensor=base.tensor, offset=base.offset + offset,
                   ap=[list(x) for x in apl])


def f32r(ap):
    return ap.bitcast(F32R)


def build_kernel(nc):
    xb = nc.dram_tensor("xb", [T, C], F32, kind="ExternalInput")
    pw = nc.dram_tensor("pw", [PATCH], F32R, kind="ExternalInput")
    wqT_d = nc.dram_tensor("wqT_d", [C, 4 * C * H], F32R, kind="ExternalInput")
    woT_d = nc.dram_tensor("woT_d", [C, H, C], F32, kind="ExternalInput")
    cosdup = nc.dram_tensor("cosdup", [128, C], F32, kind="ExternalInput")
    sinpm = nc.dram_tensor("sinpm", [128, C], F32, kind="ExternalInput")
    cmask = nc.dram_tensor("cmask", [S, S], F32, kind="ExternalInput")
    negio = nc.dram_tensor("negio", [1, NP], F32, kind="ExternalInput")
    sinkTq = nc.dram_tensor("sinkTq", [C, H], F32, kind="ExternalInput")
    sinkTk = nc.dram_tensor("sinkTk", [C, H], F32, kind="ExternalInput")
    sinkvb = nc.dram_tensor("sinkvb", [1, H * C], U16, kind="ExternalInput")
    sel16 = nc.dram_tensor("sel16", [5, NSEL], F32, kind="ExternalInput")
    onesd = nc.dram_tensor("onesd", [1, 128], F32R, kind="ExternalInput")
    ident = nc.dram_tensor("ident", [128, 128], F32, kind="ExternalInput")
    out = nc.dram_tensor("out", [NSEL, C], F32, kind="ExternalOutput")

    with tile.TileContext(nc) as tc:
        _emit(tc, nc, xb, pw, wqT_d, woT_d, cosdup, sinpm, cmask,
              negio, sinkTq, sinkTk, sinkvb, sel16, onesd, ident, out)
    return nc


def _emit(tc, nc, xb, pw, wqT_d, woT_d, cosdup, sinpm, cmask,
          negio, sinkTq, sinkTk, sinkvb, sel16, onesd, ident, out):
    import os
    LEVEL = int(os.environ.get("KLEVEL", "9"))
    from contextlib import ExitStack
    ctx = ExitStack()
    with ctx:
        const1 = ctx.enter_context(tc.tile_pool(name="const1", bufs=1))
        xpool = ctx.enter_context(tc.tile_pool(name="xpool", bufs=1))
        junkp = ctx.enter_context(tc.tile_pool(name="junkp", bufs=1))
        stat = ctx.enter_context(tc.tile_pool(name="stat", bufs=4))
        sb = ctx.enter_context(tc.tile_pool(name="sb", bufs=1))
        psall = ctx.enter_context(tc.tile_pool(name="psall", bufs=1,
                                               space="PSUM"))
        # one tile owning all 8 PSUM banks; regions choreographed manually
        PS = psall.tile([128, 4096], F32)
        # region plan (f32 cols):
        #   0:1024     qk stack (q rows 0:64, k rows 64:128); later att
        #              [65, 520] at 0:520; later out [64, 128]
        #   1024:2048  v rows 0:64; later pT staging [65, 260] at 1024:1284
        #   2048:3072  g rows 0:64
        #   2560:3584  y [65, 1024] (after sigg consumed g)
        #   3072:3584  staging A (pw bcast, x_selT, qnT)
        #   3584:4096  logits row (rows 0:1) / knT staging / ygT staging
        LROW = 3584

        # ---------------- x tile 0 first, then tiny tables ----------------
        def xdma(i):
            xp = xpool.tile([128, PATCH], F32, tag=f"xp{i}")
            nc.sync.dma_start(
                out=xp[:, :],
                in_=rap(xb[:, :], [[PATCH, 128], [1, PATCH]],
                        offset=i * 128 * PATCH))
            return xp

        xps = [xdma(0)]
        ident_t = const1.tile([128, 128], F32)
        nc.sync.dma_start(out=ident_t[:, :], in_=ident[:, :])

        pw_sb = const1.tile([1, PATCH], F32R)
        nc.sync.dma_start(out=pw_sb[:, :], in_=rap(pw[:], [[1, 1], [1, PATCH]]))
        ones_t = const1.tile([1, 128], F32R)
        nc.sync.dma_start(out=ones_t[:, :], in_=onesd[:, :])
        eps_t = const1.tile([128, 1], F32)
        nc.vector.memset(eps_t[:, :], EPS)

        # ---------------- remaining x tiles ----------------
        for i in range(1, 4):
            xps.append(xdma(i))
        cosdup_t = const1.tile([128, C], F32)
        nc.sync.dma_start(out=cosdup_t[:, :], in_=cosdup[:, :])
        sinpm_t = const1.tile([128, C], F32)
        nc.sync.dma_start(out=sinpm_t[:, :], in_=sinpm[:, :])
        cmask_t = const1.tile([S, S], F32)
        nc.sync.dma_start(out=cmask_t[:, :], in_=cmask[:, :])
        negio_t = const1.tile([1, NP], F32)
        nc.sync.dma_start(out=negio_t[:, :], in_=negio[:, :])
        sinkTq_t = const1.tile([C, H], F32)
        nc.sync.dma_start(out=sinkTq_t[:, :], in_=sinkTq[:, :])
        sinkTk_t = const1.tile([C, H], F32)
        nc.sync.dma_start(out=sinkTk_t[:, :], in_=sinkTk[:, :])
        sel16_t = const1.tile([5, NSEL], F32)
        nc.sync.dma_start(out=sel16_t[:, :], in_=sel16[:, :])
        rhs5 = const1.tile([5, 1], F32)
        nc.vector.memset(rhs5[:, :], 1.0)

        # v sink row: host-rounded bf16 bits straight into v_sb row 64
        v_sb = sb.tile([S, H, C], BF16, tag="v_sb")
        nc.sync.dma_start(
            out=v_sb[NSEL:S, :, :],
            in_=sinkvb[:, :].bitcast(BF16).rearrange(
                "p (h c) -> p h c", h=H))

        wqT = const1.tile([C, 4 * C * H], F32R)
        for wch in range(4):
            nc.sync.dma_start(out=wqT[:, 1024 * wch:1024 * (wch + 1)],
                              in_=wqT_d[:, 1024 * wch:1024 * (wch + 1)])
        woT = const1.tile([C, H, C], F32)


        # preload the sqrt activation table while ACT is idle
        dummy = stat.tile([1, 1], F32)
        nc.vector.memset(dummy[:, :], 1.0)
        nc.scalar.activation(out=dummy[:, :], in_=dummy[:, :],
                             func=AF.Sqrt)

        woTb = const1.tile([C, H, C], BF16)

        # pw broadcast to 128 partitions via K=1 matmul into staging banks
        # (two tiny warmups first lift PE off the cold p-state)
        nc.tensor.matmul(out=PS[0:128, 3071:3072], lhsT=ident_t[:, :],
                         rhs=ident_t[:, 0:1], start=True, stop=True)
        nc.tensor.matmul(out=PS[0:128, 3071:3072], lhsT=ident_t[:, :],
                         rhs=ident_t[:, 0:1], start=True, stop=True)
        pwB = const1.tile([128, PATCH], F32)
        for q4 in range(4):
            base = 3072 + 512 * (q4 % 2)
            nc.tensor.matmul(out=PS[:, base:base + 512],
                             lhsT=ones_t[:, :],
                             rhs=pw_sb[:, 512 * q4:512 * (q4 + 1)],
                             start=True, stop=True)
            nc.scalar.copy(out=pwB[:, 512 * q4:512 * (q4 + 1)],
                           in_=PS[:, base:base + 512])

        # ---------------- phase 1: per-patch stats ----------------
        junk = junkp.tile([128, PATCH], F32, tag="junk")
        junk2 = junkp.tile([128, PATCH], F32, tag="junk2")
        ss_c = stat.tile([128, 4], F32, tag="ss_c")
        dot_c = stat.tile([128, 4], F32, tag="dot_c")
        rs_c = stat.tile([128, 4], F32, tag="rs_c")
        logit_c = stat.tile([128, 4], F32, tag="logit_c")
        for i in range(4):
            xp = xps[i]
            nc.scalar.activation(out=junk[:, :], in_=xp[:, :], func=AF.Square,
                                 accum_out=ss_c[:, i:i + 1])
            nc.vector.scalar_tensor_tensor(
                out=junk2[:, :], in0=xp[:, :], scalar=1.0, in1=pwB[:, :],
                op0=ALU.mult, op1=ALU.mult, accum_out=dot_c[:, i:i + 1])
            nc.scalar.activation(out=rs_c[:, i:i + 1], in_=ss_c[:, i:i + 1],
                                 func=AF.Sqrt, bias=eps_t[:, :],
                                 scale=1.0 / PATCH)
            nc.vector.reciprocal(out=rs_c[:, i:i + 1], in_=rs_c[:, i:i + 1])
            nc.vector.tensor_mul(logit_c[:, i:i + 1], dot_c[:, i:i + 1],
                                 rs_c[:, i:i + 1])
            # transpose this tile's logits column into the PSUM row
            nc.tensor.transpose(
                out=PS[0:1, LROW + 128 * i:LROW + 128 * (i + 1)],
                in_=logit_c[:, i:i + 1], identity=ident_t[:, :])

        if LEVEL == 1:
            lrow_sb = stat.tile([1, NP], F32, tag="lrow_sb")
            nc.vector.tensor_copy(out=lrow_sb[:, :],
                                  in_=PS[0:1, LROW:LROW + NP])
            for r in range(4):
                nc.sync.dma_start(out=out[r:r + 1, :],
                                  in_=lrow_sb[0:1, 128 * r:128 * (r + 1)])
            return

        # ---------------- top-4 selection (on the PSUM row) ----------------
        lrow = PS[0:1, LROW:LROW + NP]
        max8 = stat.tile([1, 8], F32, tag="max8")
        nc.vector.max(out=max8[:, :], in_=lrow)
        masked = stat.tile([1, NP], F32, tag="masked")
        nc.vector.scalar_tensor_tensor(
            out=masked[:, :], in0=lrow, scalar=max8[:, 3:4],
            in1=negio_t[:, :], op0=ALU.is_ge, op1=ALU.mult)
        mm8 = stat.tile([1, 8], F32, tag="mm8")
        nc.vector.max(out=mm8[:, :], in_=masked[:, :])
        idx8 = stat.tile([1, 8], U32, tag="idx8")
        nc.vector.max_index(out=idx8[:, :], in_max=mm8[:, :],
                            in_values=masked[:, :])
        idxf = stat.tile([1, 8], F32, tag="idxf")
        nc.vector.tensor_copy(out=idxf[:, :], in_=idx8[:, :])

        # patch-id column via PE transpose: [1,4] -> [4,1], then token
        # ids 16*I[p] + 4T + t in (T, p, t) row order via sel16
        nc.tensor.transpose(out=PS[0:4, 3582:3583], in_=idxf[0:1, 0:4],
                            identity=ident_t[0:1, 0:1])
        nc.scalar.copy(out=rhs5[0:4, :], in_=PS[0:4, 3582:3583])
        nc.tensor.matmul(out=PS[0:NSEL, 3583:3584], lhsT=sel16_t[:, :],
                         rhs=rhs5[:, :], start=True, stop=True)
        idc_f = stat.tile([NSEL, 1], F32, tag="idc_f")
        nc.scalar.copy(out=idc_f[:, :], in_=PS[0:NSEL, 3583:3584])
        idc_i = stat.tile([NSEL, 1], I32, tag="idc_i")
        nc.vector.tensor_copy(out=idc_i[:, :], in_=idc_f[:, :])

        # gather the 64 tokens (row 16T+4p+t = token 16*I[p] + 4T + t)
        x_sel = sb.tile([NSEL, C], F32, tag="x_sel")
        nc.gpsimd.indirect_dma_start(
            out=x_sel[:, :], out_offset=None, in_=xb[:, :],
            in_offset=bass.IndirectOffsetOnAxis(ap=idc_i[:, 0:1], axis=0))

        if LEVEL == 2:
            nc.sync.dma_start(out=out[:, :], in_=x_sel[:, :])
            return

        # ---------------- qkvg projection ----------------
        nc.tensor.transpose(out=PS[0:128, 3072:3072 + NSEL], in_=x_sel[:, :],
                            identity=ident_t[0:NSEL, 0:NSEL])
        x_selT = sb.tile([C, NSEL], F32R, tag="x_selT")
        nc.scalar.copy(out=x_selT[:, :], in_=PS[:, 3072:3072 + NSEL])

        # qkvg[token, f] for the 64 gathered tokens -> PS rows 0:64
        for g in range(8):
            nc.tensor.matmul(out=PS[0:NSEL, 512 * g:512 * (g + 1)],
                             lhsT=x_selT[:, :],
                             rhs=wqT[:, 512 * g:512 * (g + 1)],
                             start=True, stop=True)
        qkvg_sb = sb.tile([NSEL, 4 * C * H], BF16, tag="qkvg_sb")
        nc.scalar.copy(out=qkvg_sb[:, 0:1024], in_=PS[0:NSEL, 0:1024])
        nc.vector.tensor_copy(out=qkvg_sb[:, 1024:2048],
                              in_=PS[0:NSEL, 1024:2048])
        nc.scalar.copy(out=qkvg_sb[:, 2048:3072], in_=PS[0:NSEL, 2048:3072])
        nc.vector.tensor_copy(out=qkvg_sb[:, 3072:4096],
                              in_=PS[0:NSEL, 3072:4096])

        # rearrange token-major -> s-major via SBUF->SBUF DMAs.
        # qkvg row 16T+4p+t (token 16*I[p]+4T+t), col (b,h,c) feeds
        # s-row 16p+4t+b of tensor T: per tensor the source rows are the
        # contiguous block 16T:16T+16 -> clean single-stride APs.
        qk = sb.tile([128, H, C], BF16, tag="qk")
        vg = sb.tile([128, H, C], BF16, tag="vg")
        FQ = 4 * C * H

        def rearr(tens, dst, half, eng):
            eng.dma_start(
                out=dst[64 * half:64 * half + NSEL, :, :],
                in_=rap(qkvg_sb[:, :], [[FQ, T0], [1024, 4], [1, 1024]],
                        offset=T0 * tens * FQ))

        rearr(0, qk, 0, nc.sync)    # q on the SP queue
        rearr(1, qk, 1, nc.scalar)  # k on the ACT queue (packs transfers)

        # out-projection weights arrive late; the dummy write makes the DMA
        # wait for the gather so it cannot block the gather's transfer
        nc.vector.tensor_copy(out=woT[0:1, 0, 0:1], in_=x_sel[0:1, 0:1])
        nc.sync.dma_start(out=woT[:, :, :], in_=woT_d[:, :, :])
        nc.gpsimd.tensor_copy(out=woTb[:, :, :], in_=woT[:, :, :])

        if LEVEL == 3:
            q0 = sb.tile([NSEL, C], F32, tag="q0dbg")
            nc.vector.tensor_copy(out=q0[:, :], in_=qk[0:NSEL, 0, :])
            nc.sync.dma_start(out=out[:, :], in_=q0[:, :])
            return

        # ---------------- rmsnorm + rope on the qk stack ----------------
        ssq = sb.tile([128, H], F32, tag="ssq")
        sqj = junkp.tile([128, H, C], F32, tag="sqj")
        nc.gpsimd.tensor_tensor(out=sqj[:, 5:8, :], in0=qk[:, 5:8, :],
                                in1=qk[:, 5:8, :], op=ALU.mult)
        nc.vector.tensor_tensor(out=sqj[:, 0:5, :], in0=qk[:, 0:5, :],
                                in1=qk[:, 0:5, :], op=ALU.mult)
        nc.vector.tensor_reduce(out=ssq[:, 0:5], in_=sqj[:, 0:5, :],
                                axis=AX.X, op=ALU.add)
        nc.vector.tensor_reduce(out=ssq[:, 5:8], in_=sqj[:, 5:8, :],
                                axis=AX.X, op=ALU.add)
        # Release the v/g rearranges only after the reduce so sigmoid's ACT
        # table load cannot be scheduled ahead of the rope sqrt: a genuine
        # RAW chain -- zro = 0*ssq, then corner += zro on one element of
        # each of the v/g source row blocks (value-preserving).
        zro = stat.tile([17, 1], F32, tag="zro")
        nc.vector.tensor_scalar_mul(zro[:, :], ssq[0:17, 0:1], 0.0)
        corner = rap(qkvg_sb[:, :], [[FQ, 17], [1, 1]], offset=32 * FQ)
        nc.vector.tensor_scalar(out=corner, in0=corner,
                                scalar1=zro[:, 0:1], scalar2=None,
                                op0=ALU.add)
        rearr(2, vg, 0, nc.sync)
        rearr(3, vg, 1, nc.sync)
        rf = sb.tile([128, H], F32, tag="rf")
        nc.scalar.activation(out=rf[:, :], in_=ssq[:, :], func=AF.Sqrt,
                             bias=eps_t[:, :], scale=1.0 / C)
        nc.vector.reciprocal(out=rf[:, :], in_=rf[:, :])
        # v -> bf16 for the y matmul; sigmoid gate from the g half (the
        # table load + sigg hide under the DVE rope chain; exp reloads
        # ln/exp before the softmax)
        nc.scalar.copy(out=v_sb[0:NSEL, :, :], in_=vg[0:NSEL, :, :])
        sigg = sb.tile([NSEL, H, C], BF16, tag="sigg")
        nc.scalar.activation(out=sigg[:, :, :], in_=vg[NSEL:128, :, :],
                             func=AF.Sigmoid)
        qk1 = sb.tile([128, H, C], F32, tag="qk1")
        r1 = sb.tile([128, H, C], F32, tag="r1")
        r2 = sb.tile([128, H, C], F32, tag="r2")
        qkn = sb.tile([128, H, C], F32, tag="qkn")

        def hs(eng, hs0, hs1):
            hn = hs1 - hs0
            eng.tensor_tensor(
                out=qk1[:, hs0:hs1, :], in0=qk[:, hs0:hs1, :],
                in1=rf[:, hs0:hs1].rearrange("p (h a) -> p h a", a=1)
                    .to_broadcast([128, hn, C]), op=ALU.mult)
            eng.tensor_tensor(
                out=r1[:, hs0:hs1, :], in0=qk1[:, hs0:hs1, :],
                in1=cosdup_t[:, :].rearrange("p (a c) -> p a c", a=1)
                    .to_broadcast([128, hn, C]), op=ALU.mult)
            eng.tensor_tensor(
                out=r2[:, hs0:hs1, 0:64], in0=qk1[:, hs0:hs1, 64:128],
                in1=sinpm_t[:, 0:64].rearrange("p (a c) -> p a c", a=1)
                    .to_broadcast([128, hn, 64]), op=ALU.mult)
            eng.tensor_tensor(
                out=r2[:, hs0:hs1, 64:128], in0=qk1[:, hs0:hs1, 0:64],
                in1=sinpm_t[:, 64:128].rearrange("p (a c) -> p a c", a=1)
                    .to_broadcast([128, hn, 64]), op=ALU.mult)
            eng.tensor_add(out=qkn[:, hs0:hs1, :], in0=r1[:, hs0:hs1, :],
                           in1=r2[:, hs0:hs1, :])

        hs(nc.vector, 0, 5)
        hs(nc.gpsimd, 5, 8)

        if LEVEL == 4:
            qn32 = sb.tile([NSEL, C], F32, tag="qn32")
            nc.vector.tensor_copy(out=qn32[:, :], in_=qkn[0:NSEL, 0, :])
            nc.sync.dma_start(out=out[:, :], in_=qn32[:, :])
            return

        # ---------------- transposes to qnT / knT ----------------
        # per head-group so group-0 attention starts while group-1 is
        # still transposing; sink columns inserted up front
        qnT = sb.tile([C, H, S], BF16, tag="qnT")
        knT = sb.tile([C, H, S], BF16, tag="knT")
        nc.scalar.copy(out=rap(qnT[:, :, :], [[H * S, C], [S, H], [1, 1]],
                               offset=NSEL),
                       in_=sinkTq_t[:, :].rearrange("c (h a) -> c h a", a=1))
        nc.scalar.copy(out=rap(knT[:, :, :], [[H * S, C], [S, H], [1, 1]],
                               offset=NSEL),
                       in_=sinkTk_t[:, :].rearrange("c (h a) -> c h a", a=1))
        for g in range(2):
            for si, dstT in enumerate((qnT, knT)):
                base = 3072 + 256 * (2 * g + si)
                for j in range(4):
                    h = 4 * g + j
                    nc.tensor.transpose(
                        out=PS[:, base + NSEL * j:base + NSEL * (j + 1)],
                        in_=qkn[64 * si:64 * (si + 1), h, :],
                        identity=ident_t[64 * si:64 * si + NSEL,
                                         64 * si:64 * si + NSEL])
                dst = rap(dstT[:, :, :], [[H * S, C], [S, 4], [1, NSEL]],
                          offset=4 * g * S)
                if si == 0:
                    nc.vector.tensor_copy(
                        out=dst, in_=PS[:, base:base + 256].rearrange(
                            "p (h s) -> p h s", h=4))
                else:
                    nc.scalar.copy(
                        out=dst, in_=PS[:, base:base + 256].rearrange(
                            "p (h s) -> p h s", h=4))

        # ---------------- attention ----------------
        # att head slots padded to 128 cols (matmul must not cross banks);
        # the whole tail runs as two independent head-group pipelines so
        # PE/DVE/ACT overlap across groups
        t0 = sb.tile([S, H, S], F32, tag="t0")
        p_sb = sb.tile([S, H, S], F32, tag="p_sb")
        den8 = sb.tile([S, H], F32, tag="den8")
        rden = sb.tile([S, H], F32, tag="rden")
        sigrd = sb.tile([NSEL, H, C], F32, tag="sigrd")
        pT = sb.tile([S, H, S], BF16, tag="pT")
        ygT = sb.tile([C, H, NSEL], BF16, tag="ygT")
        sgT_sb = sb.tile([C, H, NSEL], BF16, tag="sgT_sb")
        for g in range(2):
            hs = slice(4 * g, 4 * (g + 1))
            for h in range(4 * g, 4 * (g + 1)):
                nc.tensor.matmul(out=PS[0:S, C * h:C * h + S],
                                 lhsT=qnT[:, h, :], rhs=knT[:, h, :],
                                 start=True, stop=True)
            attg = rap(PS[:, :], [[4096, S], [C, 4], [1, S]],
                       offset=4 * g * C)
            nc.vector.tensor_tensor(
                out=t0[:, hs, :], in0=attg,
                in1=cmask_t[:, :].rearrange("s (a t) -> s a t", a=1)
                    .to_broadcast([S, 4, S]), op=ALU.add)
            nc.scalar.activation(out=p_sb[:, hs, :], in_=t0[:, hs, :],
                                 func=AF.Exp, scale=SCALE)
            nc.vector.tensor_reduce(out=den8[:, hs], in_=p_sb[:, hs, :],
                                    axis=AX.X, op=ALU.add)
            nc.vector.reciprocal(out=rden[:, hs], in_=den8[:, hs])
            # 1/den folds into the gate; pT/y consume UNNORMALIZED p
            nc.vector.tensor_tensor(
                out=sigrd[:, hs, :], in0=sigg[:, hs, :],
                in1=rden[0:NSEL, hs].rearrange("s (h a) -> s h a", a=1)
                    .to_broadcast([NSEL, 4, C]), op=ALU.mult)
            for j in range(4):
                nc.tensor.transpose(
                    out=PS[0:S, 1024 + 520 * g + S * j:
                           1024 + 520 * g + S * (j + 1)],
                    in_=p_sb[:, 4 * g + j, :], identity=ident_t[0:S, 0:S])
            nc.vector.tensor_copy(
                out=pT[:, hs, :],
                in_=PS[0:S, 1024 + 520 * g:1024 + 520 * g + 4 * S]
                    .rearrange("p (a b) -> p a b", a=4))
            # yT = v^T @ p per head (swapped operands) -> [c, s] slots
            for h in range(4 * g, 4 * (g + 1)):
                nc.tensor.matmul(out=PS[0:C, 2560 + C * h:2560 + C * h + S],
                                 lhsT=v_sb[:, h, :], rhs=pT[:, h, :],
                                 start=True, stop=True)
            # transpose the gate into [c, h, s] during the same window
            for h in range(4 * g, 4 * (g + 1)):
                nc.tensor.transpose(
                    out=PS[:, LROW + NSEL * h:LROW + NSEL * (h + 1)],
                    in_=sigrd[:, h, :], identity=ident_t[0:NSEL, 0:NSEL])
            nc.scalar.copy(
                out=rap(sgT_sb[:, :, :],
                        [[H * NSEL, C], [NSEL, 4], [1, NSEL]],
                        offset=4 * g * NSEL),
                in_=PS[:, LROW + 256 * g:LROW + 256 * (g + 1)].rearrange(
                    "p (h s) -> p h s", h=4))
            yTg = rap(PS[:, :], [[4096, C], [C, 4], [1, NSEL]],
                      offset=2560 + 4 * g * C)
            nc.vector.tensor_tensor(
                out=rap(ygT[:, :, :], [[H * NSEL, C], [NSEL, 4], [1, NSEL]],
                        offset=4 * g * NSEL),
                in0=yTg,
                in1=rap(sgT_sb[:, :, :],
                        [[H * NSEL, C], [NSEL, 4], [1, NSEL]],
                        offset=4 * g * NSEL), op=ALU.mult)

        if LEVEL == 5:
            yg32 = sb.tile([NSEL, C], F32, tag="yg32")
            nc.vector.tensor_copy(out=yg32[:, :], in_=ygT[0:NSEL, 0, :])
            nc.sync.dma_start(out=out[:, :], in_=yg32[:, :])
            return

        # ---------------- output projection ----------------
        out_sb = sb.tile([NSEL, C], F32, tag="out_sb")
        for half in range(2):
            cols = slice(64 * half, 64 * (half + 1))
            out_ps = PS[0:NSEL, 64 * half:64 * (half + 1)]
            for h in range(H):
                nc.tensor.matmul(out=out_ps, lhsT=ygT[:, h, :],
                                 rhs=woTb[:, h, cols], start=(h == 0),
                                 stop=(h == H - 1))
            nc.scalar.copy(out=out_sb[:, cols], in_=out_ps)
            nc.sync.dma_start(out=out[:, cols], in_=out_sb[:, cols])


def make_host_constants(inputs):
    """Host-side prep of tables derived from the (full) inputs."""
    cos = np.asarray(inputs["cos"]).reshape(S, 64).astype(np.float32)
    sin = np.asarray(inputs["sin"]).reshape(S, 64).astype(np.float32)
    sink = np.asarray(inputs["sink"]).reshape(H, C).astype(np.float32)
    tao = np.asarray(inputs["tao"]).astype(np.float32)
    wq = np.asarray(inputs["W_qkvg"]).astype(np.float32)
    wo = np.asarray(inputs["W_out"]).astype(np.float32)

    # partition p (0..63 in each half) holds position p+1; rows duplicated
    # for the q half (0:64) and k half (64:128)
    pos = np.arange(64) + 1
    cos_p = cos[pos]
    sin_p = sin[pos]
    cosdup = np.tile(np.concatenate([cos_p, cos_p], axis=1), (2, 1))
    sinpm = np.tile(np.concatenate([sin_p, -sin_p], axis=1), (2, 1))
    # tao folds into the rope tables: qn = (qk*rf)*cos' + swap(qk*rf)*sin'
    taocol = np.concatenate([np.full((64, 1), tao[0], np.float32),
                             np.full((64, 1), tao[1], np.float32)])
    cosdup = cosdup * taocol
    sinpm = sinpm * taocol

    # additive causal mask in s-major layout (row/col 64 = sink, pos 0)
    posf = np.where(np.arange(S) < NSEL, np.arange(S) + 1, 0)
    cmaskm = np.where(posf[None, :] <= posf[:, None], 0.0,
                      NEG_BIG).astype(np.float32)
    negio = (float(NP) - np.arange(NP, dtype=np.float32)).reshape(1, NP)

    # sink rows: rope at position 0 is identity; rmsnorm + tao on host
    sn = sink / np.sqrt((sink * sink).mean(axis=-1, keepdims=True) + EPS)
    sinkTq = np.ascontiguousarray((sn * tao[0]).T)
    sinkTk = np.ascontiguousarray((sn * tao[1]).T)
    # v sink row as bf16 bit pattern (round-to-nearest-even)
    f = sink.reshape(1, H * C).astype(np.float32)
    u = f.view(np.uint32)
    rounded = ((u + 0x7FFF + ((u >> 16) & 1)) >> 16).astype(np.uint16)
    sinkvb = np.ascontiguousarray(rounded)

    # token ids: row 16T+4p+t gathers token 16*I[p] + 4T + t
    # sel16[j, r] = 16*(j==p(r)) for j<4; sel16[4, r] = 4T(r) + t(r)
    sel16m = np.zeros((5, NSEL), np.float32)
    for Tn in range(4):
        for p in range(4):
            for t in range(4):
                r = 16 * Tn + 4 * p + t
                sel16m[p, r] = 16.0
                sel16m[4, r] = float(4 * Tn + t)

    wqT = np.ascontiguousarray(wq.T)
    woT = np.ascontiguousarray(wo.reshape(C, H, C).transpose(2, 1, 0))

    ident = np.eye(128, dtype=np.float32)
    return dict(cosdup=np.ascontiguousarray(cosdup, dtype=np.float32),
                sinpm=np.ascontiguousarray(sinpm, dtype=np.float32),
                cmask=cmaskm, negio=negio,
                sinkTq=sinkTq, sinkTk=sinkTk, sinkvb=sinkvb, sel16=sel16m,
                onesd=np.ones((1, 128), np.float32),
                wqT_d=wqT, woT_d=woT, ident=ident)


_CACHE = {}


def get_nc():
    if "nc" not in _CACHE:
        nc = bacc.Bacc("TRN2", target_bir_lowering=False, debug=False,
                       num_devices=B)
        build_kernel(nc)
        nc.compile()
        _CACHE["nc"] = nc
    return _CACHE["nc"]


def make_in_maps(inputs):
    x = np.ascontiguousarray(inputs["x"], dtype=np.float32)
    pwv = np.ascontiguousarray(inputs["patch_w"], dtype=np.float32)
    consts = make_host_constants(inputs)
    in_maps = []
    for b in range(B):
        m = {"xb": np.ascontiguousarray(x[b]), "pw": pwv}
        m.update(consts)
        in_maps.append(m)
    return in_maps


def kernel(**inputs):
    nc = get_nc()
    in_maps = make_in_maps(inputs)
    res = run_bass_kernel_spmd(nc, in_maps, core_ids=list(range(B)))
    return np.stack([r["out"] for r in res.results], axis=0)


if __name__ == "__main__":
    nc = get_nc()
    print("build ok:", len(nc.m.functions[0].allocations), "allocations")


# revision 42
# speedup vs baseline: 2.1719x; 1.0337x over previous
"""Trainium2 Bass kernel for nn_AttentionOnDetail (sparse patch attention).

Data-parallel over batch B=8 across 8 NeuronCores; one batch per core.

v2 design (latency-focused; the kernel is dependency-bound, not
throughput-bound):
  - Host-side prep inside kernel(): W_qkvg.T / W_out.T passed
    pre-transposed, cos/sin tables pre-permuted and duplicated,
    rmsnorm(sink)*tao rows precomputed (rope at position 0 is identity).
  - x tile DMAs issued first; patch stats (ACT square+accum, DVE dot)
    pipeline behind them; per-tile logits transposed into a PSUM row via
    PE so top-4 selection needs no DMA.
  - Top-4 via max8/threshold/max_index (ascending patch order); token
    gather via one indirect DMA of 4 whole patches.
  - qkvg projection computed "PE-direct": per (tensor, kilo-block)
    matmuls with column-selected lhsT views and stride-4 PSUM output
    rows land q/k/v/g directly in attention layout (no DRAM bounce, no
    rearrange DMAs).  fp32r operands -> 1 cycle/row.
  - q and k stacked on 128 partitions: rmsnorm+rope for both costs one
    set of full-width DVE ops (cost scales with free size only).
    rsqrt via ln+exp keeps ACT on a single function table; tao folded
    into the exp bias.
  - attention: bf16 matmuls, no row-max softmax (range is bounded),
    one exp over all heads, 1/den folded into p before the transpose.
"""

import sys
import numpy as np

for _p in ("/opt/trn_rl_repo",):
    if _p not in sys.path:
        sys.path.insert(0, _p)

import concourse.bass as bass
import concourse.bacc as bacc
import concourse.tile as tile
from concourse import mybir
from concourse.bass_utils import run_bass_kernel_spmd

F32 = mybir.dt.float32
F32R = mybir.dt.float32r
BF16 = mybir.dt.bfloat16
I32 = mybir.dt.int32
U32 = mybir.dt.uint32
U16 = mybir.dt.uint16
AF = mybir.ActivationFunctionType
ALU = mybir.AluOpType
AX = mybir.AxisListType

B, T, C, H, T0 = 8, 8192, 128, 8, 16
NP = T // T0          # 512 patches
PATCH = T0 * C        # 2048 elements per patch
S = 65                # sink + 64 selected tokens
NSEL = 64
EPS = 1.1920929e-07
SCALE = 1.0 / float(np.sqrt(np.float32(C)))
NEG_BIG = -1.0e30


def rap(t, apl, offset=0):
    """Raw AP over a tile/AP's storage, flat element strides."""
    base = t if isinstance(t, bass.AP) else t[:]
    return bass.AP(tensor=base.tensor, offset=base.offset + offset,
                   ap=[list(x) for x in apl])


def f32r(ap):
    return ap.bitcast(F32R)


def build_kernel(nc):
    xb = nc.dram_tensor("xb", [T, C], F32, kind="ExternalInput")
    pw = nc.dram_tensor("pw", [PATCH], F32R, kind="ExternalInput")
    wqT_d = nc.dram_tensor("wqT_d", [C, 4 * C * H], F32R, kind="ExternalInput")
    woT_d = nc.dram_tensor("woT_d", [C, H, C], F32, kind="ExternalInput")
    cosdup = nc.dram_tensor("cosdup", [128, C], F32, kind="ExternalInput")
    sinpm = nc.dram_tensor("sinpm", [128, C], F32, kind="ExternalInput")
    cmask = nc.dram_tensor("cmask", [S, S], F32, kind="ExternalInput")
    negio = nc.dram_tensor("negio", [1, NP], F32, kind="ExternalInput")
    sinkTq = nc.dram_tensor("sinkTq", [C, H], F32, kind="ExternalInput")
    sinkTk = nc.dram_tensor("sinkTk", [C, H], F32, kind="ExternalInput")
    sinkvb = nc.dram_tensor("sinkvb", [1, H * C], U16, kind="ExternalInput")
    sel16 = nc.dram_tensor("sel16", [5, NSEL], F32, kind="ExternalInput")
    onesd = nc.dram_tensor("onesd", [1, 128], F32R, kind="ExternalInput")
    ident = nc.dram_tensor("ident", [128, 128], F32, kind="ExternalInput")
    out = nc.dram_tensor("out", [NSEL, C], F32, kind="ExternalOutput")

    with tile.TileContext(nc) as tc:
        _emit(tc, nc, xb, pw, wqT_d, woT_d, cosdup, sinpm, cmask,
              negio, sinkTq, sinkTk, sinkvb, sel16, onesd, ident, out)
    return nc


def _emit(tc, nc, xb, pw, wqT_d, woT_d, cosdup, sinpm, cmask,
          negio, sinkTq, sinkTk, sinkvb, sel16, onesd, ident, out):
    import os
    LEVEL = int(os.environ.get("KLEVEL", "9"))
    from contextlib import ExitStack
    ctx = ExitStack()
    with ctx:
        const1 = ctx.enter_context(tc.tile_pool(name="const1", bufs=1))
        xpool = ctx.enter_context(tc.tile_pool(name="xpool", bufs=1))
        junkp = ctx.enter_context(tc.tile_pool(name="junkp", bufs=1))
        stat = ctx.enter_context(tc.tile_pool(name="stat", bufs=4))
        sb = ctx.enter_context(tc.tile_pool(name="sb", bufs=1))
        psall = ctx.enter_context(tc.tile_pool(name="psall", bufs=1,
                                               space="PSUM"))
        # one tile owning all 8 PSUM banks; regions choreographed manually
        PS = psall.tile([128, 4096], F32)
        # region plan (f32 cols):
        #   0:1024     qk stack (q rows 0:64, k rows 64:128); later att
        #              [65, 520] at 0:520; later out [64, 128]
        #   1024:2048  v rows 0:64; later pT staging [65, 260] at 1024:1284
        #   2048:3072  g rows 0:64
        #   2560:3584  y [65, 1024] (after sigg consumed g)
        #   3072:3584  staging A (pw bcast, x_selT, qnT)
        #   3584:4096  logits row (rows 0:1) / knT staging / ygT staging
        LROW = 3584

        # ---------------- x tile 0 first, then tiny tables ----------------
        def xdma(i):
            xp = xpool.tile([128, PATCH], F32, tag=f"xp{i}")
            nc.sync.dma_start(
                out=xp[:, :],
                in_=rap(xb[:, :], [[PATCH, 128], [1, PATCH]],
                        offset=i * 128 * PATCH))
            return xp

        xps = [xdma(0)]
        ident_t = const1.tile([128, 128], F32)
        nc.sync.dma_start(out=ident_t[:, :], in_=ident[:, :])

        pw_sb = const1.tile([1, PATCH], F32R)
        nc.sync.dma_start(out=pw_sb[:, :], in_=rap(pw[:], [[1, 1], [1, PATCH]]))
        ones_t = const1.tile([1, 128], F32R)
        nc.sync.dma_start(out=ones_t[:, :], in_=onesd[:, :])
        eps_t = const1.tile([128, 1], F32)
        nc.vector.memset(eps_t[:, :], EPS)

        # ---------------- remaining x tiles ----------------
        for i in range(1, 4):
            xps.append(xdma(i))
        cosdup_t = const1.tile([128, C], F32)
        nc.sync.dma_start(out=cosdup_t[:, :], in_=cosdup[:, :])
        sinpm_t = const1.tile([128, C], F32)
        nc.sync.dma_start(out=sinpm_t[:, :], in_=sinpm[:, :])
        cmask_t = const1.tile([S, S], F32)
        nc.sync.dma_start(out=cmask_t[:, :], in_=cmask[:, :])
        negio_t = const1.tile([1, NP], F32)
        nc.sync.dma_start(out=negio_t[:, :], in_=negio[:, :])
        sinkTq_t = const1.tile([C, H], F32)
        nc.sync.dma_start(out=sinkTq_t[:, :], in_=sinkTq[:, :])
        sinkTk_t = const1.tile([C, H], F32)
        nc.sync.dma_start(out=sinkTk_t[:, :], in_=sinkTk[:, :])
        sel16_t = const1.tile([5, NSEL], F32)
        nc.sync.dma_start(out=sel16_t[:, :], in_=sel16[:, :])
        rhs5 = const1.tile([5, 1], F32)
        nc.vector.memset(rhs5[:, :], 1.0)

        # v sink row: host-rounded bf16 bits straight into v_sb row 64
        v_sb = sb.tile([S, H, C], BF16, tag="v_sb")
        nc.sync.dma_start(
            out=v_sb[NSEL:S, :, :],
            in_=sinkvb[:, :].bitcast(BF16).rearrange(
                "p (h c) -> p h c", h=H))

        wqT = const1.tile([C, 4 * C * H], F32R)
        for wch in range(4):
            nc.sync.dma_start(out=wqT[:, 1024 * wch:1024 * (wch + 1)],
                              in_=wqT_d[:, 1024 * wch:1024 * (wch + 1)])
        woT = const1.tile([C, H, C], F32)


        # preload the sqrt activation table while ACT is idle
        dummy = stat.tile([1, 1], F32)
        nc.vector.memset(dummy[:, :], 1.0)
        nc.scalar.activation(out=dummy[:, :], in_=dummy[:, :],
                             func=AF.Sqrt)

        woTb = const1.tile([C, H, C], BF16)

        # pw broadcast to 128 partitions via K=1 matmul into staging banks
        # (two tiny warmups first lift PE off the cold p-state)
        nc.tensor.matmul(out=PS[0:128, 3071:3072], lhsT=ident_t[:, :],
                         rhs=ident_t[:, 0:1], start=True, stop=True)
        nc.tensor.matmul(out=PS[0:128, 3071:3072], lhsT=ident_t[:, :],
                         rhs=ident_t[:, 0:1], start=True, stop=True)
        pwB = const1.tile([128, PATCH], F32)
        for q4 in range(4):
            base = 3072 + 512 * (q4 % 2)
            nc.tensor.matmul(out=PS[:, base:base + 512],
                             lhsT=ones_t[:, :],
                             rhs=pw_sb[:, 512 * q4:512 * (q4 + 1)],
                             start=True, stop=True)
            if q4 % 2 == 0:
                nc.scalar.copy(out=pwB[:, 512 * q4:512 * (q4 + 1)],
                               in_=PS[:, base:base + 512])
            else:
                nc.vector.tensor_copy(out=pwB[:, 512 * q4:512 * (q4 + 1)],
                                      in_=PS[:, base:base + 512])

        # ---------------- phase 1: per-patch stats ----------------
        junk = junkp.tile([128, PATCH], F32, tag="junk")
        junk2 = junkp.tile([128, PATCH], F32, tag="junk2")
        ss_c = stat.tile([128, 4], F32, tag="ss_c")
        dot_c = stat.tile([128, 4], F32, tag="dot_c")
        rs_c = stat.tile([128, 4], F32, tag="rs_c")
        logit_c = stat.tile([128, 4], F32, tag="logit_c")
        for i in range(4):
            xp = xps[i]
            nc.scalar.activation(out=junk[:, :], in_=xp[:, :], func=AF.Square,
                                 accum_out=ss_c[:, i:i + 1])
            nc.vector.scalar_tensor_tensor(
                out=junk2[:, :], in0=xp[:, :], scalar=1.0, in1=pwB[:, :],
                op0=ALU.mult, op1=ALU.mult, accum_out=dot_c[:, i:i + 1])
            nc.scalar.activation(out=rs_c[:, i:i + 1], in_=ss_c[:, i:i + 1],
                                 func=AF.Sqrt, bias=eps_t[:, :],
                                 scale=1.0 / PATCH)
            nc.vector.reciprocal(out=rs_c[:, i:i + 1], in_=rs_c[:, i:i + 1])
            nc.vector.tensor_mul(logit_c[:, i:i + 1], dot_c[:, i:i + 1],
                                 rs_c[:, i:i + 1])
            # transpose this tile's logits column into the PSUM row
            nc.tensor.transpose(
                out=PS[0:1, LROW + 128 * i:LROW + 128 * (i + 1)],
                in_=logit_c[:, i:i + 1], identity=ident_t[:, :])

        if LEVEL == 1:
            lrow_sb = stat.tile([1, NP], F32, tag="lrow_sb")
            nc.vector.tensor_copy(out=lrow_sb[:, :],
                                  in_=PS[0:1, LROW:LROW + NP])
            for r in range(4):
                nc.sync.dma_start(out=out[r:r + 1, :],
                                  in_=lrow_sb[0:1, 128 * r:128 * (r + 1)])
            return

        # ---------------- top-4 selection (on the PSUM row) ----------------
        lrow = PS[0:1, LROW:LROW + NP]
        max8 = stat.tile([1, 8], F32, tag="max8")
        nc.vector.max(out=max8[:, :], in_=lrow)
        masked = stat.tile([1, NP], F32, tag="masked")
        nc.vector.scalar_tensor_tensor(
            out=masked[:, :], in0=lrow, scalar=max8[:, 3:4],
            in1=negio_t[:, :], op0=ALU.is_ge, op1=ALU.mult)
        mm8 = stat.tile([1, 8], F32, tag="mm8")
        nc.vector.max(out=mm8[:, :], in_=masked[:, :])
        idx8 = stat.tile([1, 8], U32, tag="idx8")
        nc.vector.max_index(out=idx8[:, :], in_max=mm8[:, :],
                            in_values=masked[:, :])
        idxf = stat.tile([1, 8], F32, tag="idxf")
        nc.vector.tensor_copy(out=idxf[:, :], in_=idx8[:, :])

        # patch-id column via PE transpose: [1,4] -> [4,1], then token
        # ids 16*I[p] + 4T + t in (T, p, t) row order via sel16
        nc.tensor.transpose(out=PS[0:4, 3582:3583], in_=idxf[0:1, 0:4],
                            identity=ident_t[0:1, 0:1])
        nc.scalar.copy(out=rhs5[0:4, :], in_=PS[0:4, 3582:3583])
        nc.tensor.matmul(out=PS[0:NSEL, 3583:3584], lhsT=sel16_t[:, :],
                         rhs=rhs5[:, :], start=True, stop=True)
        idc_i = stat.tile([NSEL, 1], I32, tag="idc_i")
        nc.vector.tensor_copy(out=idc_i[:, :], in_=PS[0:NSEL, 3583:3584])

        # gather the 64 tokens (row 16T+4p+t = token 16*I[p] + 4T + t)
        x_sel = sb.tile([NSEL, C], F32, tag="x_sel")
        nc.gpsimd.indirect_dma_start(
            out=x_sel[:, :], out_offset=None, in_=xb[:, :],
            in_offset=bass.IndirectOffsetOnAxis(ap=idc_i[:, 0:1], axis=0))


        if LEVEL == 2:
            nc.sync.dma_start(out=out[:, :], in_=x_sel[:, :])
            return

        # ---------------- qkvg projection ----------------
        nc.tensor.transpose(out=PS[0:128, 3072:3072 + NSEL], in_=x_sel[:, :],
                            identity=ident_t[0:NSEL, 0:NSEL])
        x_selT = sb.tile([C, NSEL], F32R, tag="x_selT")
        nc.scalar.copy(out=x_selT[:, :], in_=PS[:, 3072:3072 + NSEL])

        # qkvg[token, f] for the 64 gathered tokens -> PS rows 0:64
        for g in range(8):
            nc.tensor.matmul(out=PS[0:NSEL, 512 * g:512 * (g + 1)],
                             lhsT=x_selT[:, :],
                             rhs=wqT[:, 512 * g:512 * (g + 1)],
                             start=True, stop=True)
        qkvg_sb = sb.tile([NSEL, 4 * C * H], BF16, tag="qkvg_sb")
        nc.scalar.copy(out=qkvg_sb[:, 0:1024], in_=PS[0:NSEL, 0:1024])
        nc.vector.tensor_copy(out=qkvg_sb[:, 1024:2048],
                              in_=PS[0:NSEL, 1024:2048])
        nc.scalar.copy(out=qkvg_sb[:, 2048:3072], in_=PS[0:NSEL, 2048:3072])
        nc.vector.tensor_copy(out=qkvg_sb[:, 3072:4096],
                              in_=PS[0:NSEL, 3072:4096])

        # rearrange token-major -> s-major via SBUF->SBUF DMAs.
        # qkvg row 16T+4p+t (token 16*I[p]+4T+t), col (b,h,c) feeds
        # s-row 16p+4t+b of tensor T: per tensor the source rows are the
        # contiguous block 16T:16T+16 -> clean single-stride APs.
        qk = sb.tile([128, H, C], BF16, tag="qk")
        vg = sb.tile([128, H, C], BF16, tag="vg")
        FQ = 4 * C * H

        def rearr(tens, dst, half, eng):
            eng.dma_start(
                out=dst[64 * half:64 * half + NSEL, :, :],
                in_=rap(qkvg_sb[:, :], [[FQ, T0], [1024, 4], [1, 1024]],
                        offset=T0 * tens * FQ))

        rearr(0, qk, 0, nc.sync)    # q on the SP queue
        rearr(1, qk, 1, nc.scalar)  # k on the ACT queue (packs transfers)

        # out-projection weights arrive late; the dummy write makes the DMA
        # wait for the gather so it cannot block the gather's transfer
        nc.vector.tensor_copy(out=woT[0:1, 0, 0:1], in_=x_sel[0:1, 0:1])
        nc.sync.dma_start(out=woT[:, :, :], in_=woT_d[:, :, :])
        nc.gpsimd.tensor_copy(out=woTb[:, :, :], in_=woT[:, :, :])

        if LEVEL == 3:
            q0 = sb.tile([NSEL, C], F32, tag="q0dbg")
            nc.vector.tensor_copy(out=q0[:, :], in_=qk[0:NSEL, 0, :])
            nc.sync.dma_start(out=out[:, :], in_=q0[:, :])
            return

        # ---------------- rmsnorm + rope on the qk stack ----------------
        ssq = sb.tile([128, H], F32, tag="ssq")
        sqj = junkp.tile([128, H, C], F32, tag="sqj")
        nc.gpsimd.tensor_tensor(out=sqj[:, 5:8, :], in0=qk[:, 5:8, :],
                                in1=qk[:, 5:8, :], op=ALU.mult)
        nc.vector.tensor_tensor(out=sqj[:, 0:5, :], in0=qk[:, 0:5, :],
                                in1=qk[:, 0:5, :], op=ALU.mult)
        # gate the v/g rearranges on the stt output so the scheduler keeps
        # sigmoid's table load behind the rope sqrt (value-preserving
        # corner write on the v/g source rows)
        zro = stat.tile([17, 1], F32, tag="zro")
        nc.vector.tensor_scalar_mul(zro[:, :], sqj[0:17, 0, 0:1], 0.0)
        corner = rap(qkvg_sb[:, :], [[FQ, 17], [1, 1]], offset=32 * FQ)
        nc.vector.tensor_scalar(out=corner, in0=corner,
                                scalar1=zro[:, 0:1], scalar2=None,
                                op0=ALU.add)
        rearr(2, vg, 0, nc.sync)
        rearr(3, vg, 1, nc.sync)
        nc.gpsimd.tensor_copy(out=v_sb[0:NSEL, :, :], in_=vg[0:NSEL, :, :])
        nc.vector.tensor_reduce(out=ssq[:, 0:5], in_=sqj[:, 0:5, :],
                                axis=AX.X, op=ALU.add)
        nc.vector.tensor_reduce(out=ssq[:, 5:8], in_=sqj[:, 5:8, :],
                                axis=AX.X, op=ALU.add)
        rf = sb.tile([128, H], F32, tag="rf")
        nc.scalar.activation(out=rf[:, :], in_=ssq[:, :], func=AF.Sqrt,
                             bias=eps_t[:, :], scale=1.0 / C)
        nc.vector.reciprocal(out=rf[:, :], in_=rf[:, :])
        # sigmoid gate now, exp loads after: both ACT table loads land in
        # the rope shadow, and the softmax exps then run load-free
        sigg = sb.tile([NSEL, H, C], BF16, tag="sigg")
        nc.scalar.activation(out=sigg[:, :, :], in_=vg[NSEL:128, :, :],
                             func=AF.Sigmoid)
        qk1 = sb.tile([128, H, C], F32, tag="qk1")
        r1 = sb.tile([128, H, C], F32, tag="r1")
        r2 = sb.tile([128, H, C], F32, tag="r2")
        qkn = sb.tile([128, H, C], F32, tag="qkn")

        def hs(eng, hs0, hs1):
            hn = hs1 - hs0
            eng.tensor_tensor(
                out=qk1[:, hs0:hs1, :], in0=qk[:, hs0:hs1, :],
                in1=rf[:, hs0:hs1].rearrange("p (h a) -> p h a", a=1)
                    .to_broadcast([128, hn, C]), op=ALU.mult)
            eng.tensor_tensor(
                out=r1[:, hs0:hs1, :], in0=qk1[:, hs0:hs1, :],
                in1=cosdup_t[:, :].rearrange("p (a c) -> p a c", a=1)
                    .to_broadcast([128, hn, C]), op=ALU.mult)
            eng.tensor_tensor(
                out=r2[:, hs0:hs1, 0:64], in0=qk1[:, hs0:hs1, 64:128],
                in1=sinpm_t[:, 0:64].rearrange("p (a c) -> p a c", a=1)
                    .to_broadcast([128, hn, 64]), op=ALU.mult)
            eng.tensor_tensor(
                out=r2[:, hs0:hs1, 64:128], in0=qk1[:, hs0:hs1, 0:64],
                in1=sinpm_t[:, 64:128].rearrange("p (a c) -> p a c", a=1)
                    .to_broadcast([128, hn, 64]), op=ALU.mult)
            eng.tensor_add(out=qkn[:, hs0:hs1, :], in0=r1[:, hs0:hs1, :],
                           in1=r2[:, hs0:hs1, :])

        hs(nc.vector, 0, 5)
        hs(nc.gpsimd, 5, 8)

        if LEVEL == 4:
            qn32 = sb.tile([NSEL, C], F32, tag="qn32")
            nc.vector.tensor_copy(out=qn32[:, :], in_=qkn[0:NSEL, 0, :])
            nc.sync.dma_start(out=out[:, :], in_=qn32[:, :])
            return

        # ---------------- transposes to qnT / knT ----------------
        # per head-group so group-0 attention starts while group-1 is
        # still transposing; sink columns inserted up front
        qnT = sb.tile([C, H, S], BF16, tag="qnT")
        knT = sb.tile([C, H, S], BF16, tag="knT")
        nc.scalar.copy(out=rap(qnT[:, :, :], [[H * S, C], [S, H], [1, 1]],
                               offset=NSEL),
                       in_=sinkTq_t[:, :].rearrange("c (h a) -> c h a", a=1))
        nc.scalar.copy(out=rap(knT[:, :, :], [[H * S, C], [S, H], [1, 1]],
                               offset=NSEL),
                       in_=sinkTk_t[:, :].rearrange("c (h a) -> c h a", a=1))
        for g in range(2):
            for si, dstT in enumerate((qnT, knT)):
                base = 3072 + 256 * (2 * g + si)
                for j in range(4):
                    h = 4 * g + j
                    nc.tensor.transpose(
                        out=PS[:, base + NSEL * j:base + NSEL * (j + 1)],
                        in_=qkn[64 * si:64 * (si + 1), h, :],
                        identity=ident_t[64 * si:64 * si + NSEL,
                                         64 * si:64 * si + NSEL])
                dst = rap(dstT[:, :, :], [[H * S, C], [S, 4], [1, NSEL]],
                          offset=4 * g * S)
                nc.vector.tensor_copy(
                    out=dst, in_=PS[:, base:base + 256].rearrange(
                        "p (h s) -> p h s", h=4))

        # ---------------- attention ----------------
        # att head slots padded to 128 cols (matmul must not cross banks);
        # the whole tail runs as two independent head-group pipelines so
        # PE/DVE/ACT overlap across groups
        t0 = sb.tile([S, H, S], F32, tag="t0")
        p_sb = sb.tile([S, H, S], F32, tag="p_sb")
        den8 = sb.tile([S, H], F32, tag="den8")
        rden = sb.tile([S, H], F32, tag="rden")
        sigrd = sb.tile([NSEL, H, C], F32, tag="sigrd")
        pT = sb.tile([S, H, S], BF16, tag="pT")
        ygT = sb.tile([C, H, NSEL], BF16, tag="ygT")
        sgT_sb = sb.tile([C, H, NSEL], BF16, tag="sgT_sb")
        for g in range(2):
            hs = slice(4 * g, 4 * (g + 1))
            for h in range(4 * g, 4 * (g + 1)):
                nc.tensor.matmul(out=PS[0:S, C * h:C * h + S],
                                 lhsT=qnT[:, h, :], rhs=knT[:, h, :],
                                 start=True, stop=True)
            attg = rap(PS[:, :], [[4096, S], [C, 4], [1, S]],
                       offset=4 * g * C)
            nc.vector.tensor_tensor(
                out=t0[:, hs, :], in0=attg,
                in1=cmask_t[:, :].rearrange("s (a t) -> s a t", a=1)
                    .to_broadcast([S, 4, S]), op=ALU.add)
            nc.scalar.activation(out=p_sb[:, hs, :], in_=t0[:, hs, :],
                                 func=AF.Exp, scale=SCALE)
            nc.vector.tensor_reduce(out=den8[:, hs], in_=p_sb[:, hs, :],
                                    axis=AX.X, op=ALU.add)
            nc.vector.reciprocal(out=rden[:, hs], in_=den8[:, hs])
            # 1/den folds into the gate; pT/y consume UNNORMALIZED p
            nc.vector.tensor_tensor(
                out=sigrd[:, hs, :], in0=sigg[:, hs, :],
                in1=rden[0:NSEL, hs].rearrange("s (h a) -> s h a", a=1)
                    .to_broadcast([NSEL, 4, C]), op=ALU.mult)
            for j in range(4):
                nc.tensor.transpose(
                    out=PS[0:S, 1024 + 520 * g + S * j:
                           1024 + 520 * g + S * (j + 1)],
                    in_=p_sb[:, 4 * g + j, :], identity=ident_t[0:S, 0:S])
            nc.scalar.copy(
                out=pT[:, hs, :],
                in_=PS[0:S, 1024 + 520 * g:1024 + 520 * g + 4 * S]
                    .rearrange("p (a b) -> p a b", a=4))
            # yT = v^T @ p per head (swapped operands) -> [c, s] slots
            for h in range(4 * g, 4 * (g + 1)):
                nc.tensor.matmul(out=PS[0:C, 2560 + C * h:2560 + C * h + S],
                                 lhsT=v_sb[:, h, :], rhs=pT[:, h, :],
                                 start=True, stop=True)
            # transpose the gate into [c, h, s] during the same window
            for h in range(4 * g, 4 * (g + 1)):
                nc.tensor.transpose(
                    out=PS[:, LROW + NSEL * h:LROW + NSEL * (h + 1)],
                    in_=sigrd[:, h, :], identity=ident_t[0:NSEL, 0:NSEL])
            nc.scalar.copy(
                out=rap(sgT_sb[:, :, :],
                        [[H * NSEL, C], [NSEL, 4], [1, NSEL]],
                        offset=4 * g * NSEL),
                in_=PS[:, LROW + 256 * g:LROW + 256 * (g + 1)].rearrange(
                    "p (h s) -> p h s", h=4))
            yTg = rap(PS[:, :], [[4096, C], [C, 4], [1, NSEL]],
                      offset=2560 + 4 * g * C)
            nc.vector.tensor_tensor(
                out=rap(ygT[:, :, :], [[H * NSEL, C], [NSEL, 4], [1, NSEL]],
                        offset=4 * g * NSEL),
                in0=yTg,
                in1=rap(sgT_sb[:, :, :],
                        [[H * NSEL, C], [NSEL, 4], [1, NSEL]],
                        offset=4 * g * NSEL), op=ALU.mult)

        if LEVEL == 5:
            yg32 = sb.tile([NSEL, C], F32, tag="yg32")
            nc.vector.tensor_copy(out=yg32[:, :], in_=ygT[0:NSEL, 0, :])
            nc.sync.dma_start(out=out[:, :], in_=yg32[:, :])
            return

        # ---------------- output projection ----------------
        out_sb = sb.tile([NSEL, C], F32, tag="out_sb")
        for half in range(2):
            cols = slice(64 * half, 64 * (half + 1))
            out_ps = PS[0:NSEL, 64 * half:64 * (half + 1)]
            for h in range(H):
                nc.tensor.matmul(out=out_ps, lhsT=ygT[:, h, :],
                                 rhs=woTb[:, h, cols], start=(h == 0),
                                 stop=(h == H - 1))
            nc.scalar.copy(out=out_sb[:, cols], in_=out_ps)
            nc.sync.dma_start(out=out[:, cols], in_=out_sb[:, cols])


def make_host_constants(inputs):
    """Host-side prep of tables derived from the (full) inputs."""
    cos = np.asarray(inputs["cos"]).reshape(S, 64).astype(np.float32)
    sin = np.asarray(inputs["sin"]).reshape(S, 64).astype(np.float32)
    sink = np.asarray(inputs["sink"]).reshape(H, C).astype(np.float32)
    tao = np.asarray(inputs["tao"]).astype(np.float32)
    wq = np.asarray(inputs["W_qkvg"]).astype(np.float32)
    wo = np.asarray(inputs["W_out"]).astype(np.float32)

    # partition p (0..63 in each half) holds position p+1; rows duplicated
    # for the q half (0:64) and k half (64:128)
    pos = np.arange(64) + 1
    cos_p = cos[pos]
    sin_p = sin[pos]
    cosdup = np.tile(np.concatenate([cos_p, cos_p], axis=1), (2, 1))
    sinpm = np.tile(np.concatenate([sin_p, -sin_p], axis=1), (2, 1))
    # tao folds into the rope tables: qn = (qk*rf)*cos' + swap(qk*rf)*sin'
    taocol = np.concatenate([np.full((64, 1), tao[0], np.float32),
                             np.full((64, 1), tao[1], np.float32)])
    cosdup = cosdup * taocol
    sinpm = sinpm * taocol

    # additive causal mask in s-major layout (row/col 64 = sink, pos 0)
    posf = np.where(np.arange(S) < NSEL, np.arange(S) + 1, 0)
    cmaskm = np.where(posf[None, :] <= posf[:, None], 0.0,
                      NEG_BIG).astype(np.float32)
    negio = (float(NP) - np.arange(NP, dtype=np.float32)).reshape(1, NP)

    # sink rows: rope at position 0 is identity; rmsnorm + tao on host
    sn = sink / np.sqrt((sink * sink).mean(axis=-1, keepdims=True) + EPS)
    sinkTq = np.ascontiguousarray((sn * tao[0]).T)
    sinkTk = np.ascontiguousarray((sn * tao[1]).T)
    # v sink row as bf16 bit pattern (round-to-nearest-even)
    f = sink.reshape(1, H * C).astype(np.float32)
    u = f.view(np.uint32)
    rounded = ((u + 0x7FFF + ((u >> 16) & 1)) >> 16).astype(np.uint16)
    sinkvb = np.ascontiguousarray(rounded)

    # token ids: row 16T+4p+t gathers token 16*I[p] + 4T + t
    # sel16[j, r] = 16*(j==p(r)) for j<4; sel16[4, r] = 4T(r) + t(r)
    sel16m = np.zeros((5, NSEL), np.float32)
    for Tn in range(4):
        for p in range(4):
            for t in range(4):
                r = 16 * Tn + 4 * p + t
                sel16m[p, r] = 16.0
                sel16m[4, r] = float(4 * Tn + t)

    wqT = np.ascontiguousarray(wq.T)
    woT = np.ascontiguousarray(wo.reshape(C, H, C).transpose(2, 1, 0))

    ident = np.eye(128, dtype=np.float32)
    return dict(cosdup=np.ascontiguousarray(cosdup, dtype=np.float32),
                sinpm=np.ascontiguousarray(sinpm, dtype=np.float32),
                cmask=cmaskm, negio=negio,
                sinkTq=sinkTq, sinkTk=sinkTk, sinkvb=sinkvb, sel16=sel16m,
                onesd=np.ones((1, 128), np.float32),
                wqT_d=wqT, woT_d=woT, ident=ident)


_CACHE = {}


def get_nc():
    if "nc" not in _CACHE:
        nc = bacc.Bacc("TRN2", target_bir_lowering=False, debug=False,
                       num_devices=B)
        build_kernel(nc)
        nc.compile()
        _CACHE["nc"] = nc
    return _CACHE["nc"]


def make_in_maps(inputs):
    x = np.ascontiguousarray(inputs["x"], dtype=np.float32)
    pwv = np.ascontiguousarray(inputs["patch_w"], dtype=np.float32)
    consts = make_host_constants(inputs)
    in_maps = []
    for b in range(B):
        m = {"xb": np.ascontiguousarray(x[b]), "pw": pwv}
        m.update(consts)
        in_maps.append(m)
    return in_maps


def kernel(**inputs):
    nc = get_nc()
    in_maps = make_in_maps(inputs)
    res = run_bass_kernel_spmd(nc, in_maps, core_ids=list(range(B)))
    return np.stack([r["out"] for r in res.results], axis=0)


if __name__ == "__main__":
    nc = get_nc()
    print("build ok:", len(nc.m.functions[0].allocations), "allocations")


# revision 46
# speedup vs baseline: 2.2331x; 1.0282x over previous
"""Trainium2 Bass kernel for nn_AttentionOnDetail (sparse patch attention).

Data-parallel over batch B=8 across 8 NeuronCores; one batch per core.

v2 design (latency-focused; the kernel is dependency-bound, not
throughput-bound):
  - Host-side prep inside kernel(): W_qkvg.T / W_out.T passed
    pre-transposed, cos/sin tables pre-permuted and duplicated,
    rmsnorm(sink)*tao rows precomputed (rope at position 0 is identity).
  - x tile DMAs issued first; patch stats (ACT square+accum, DVE dot)
    pipeline behind them; per-tile logits transposed into a PSUM row via
    PE so top-4 selection needs no DMA.
  - Top-4 via max8/threshold/max_index (ascending patch order); token
    gather via one indirect DMA of 4 whole patches.
  - qkvg projection computed "PE-direct": per (tensor, kilo-block)
    matmuls with column-selected lhsT views and stride-4 PSUM output
    rows land q/k/v/g directly in attention layout (no DRAM bounce, no
    rearrange DMAs).  fp32r operands -> 1 cycle/row.
  - q and k stacked on 128 partitions: rmsnorm+rope for both costs one
    set of full-width DVE ops (cost scales with free size only).
    rsqrt via ln+exp keeps ACT on a single function table; tao folded
    into the exp bias.
  - attention: bf16 matmuls, no row-max softmax (range is bounded),
    one exp over all heads, 1/den folded into p before the transpose.
"""

import sys
import numpy as np

for _p in ("/opt/trn_rl_repo",):
    if _p not in sys.path:
        sys.path.insert(0, _p)

import concourse.bass as bass
import concourse.bacc as bacc
import concourse.tile as tile
from concourse import mybir
from concourse.bass_utils import run_bass_kernel_spmd

F32 = mybir.dt.float32
F32R = mybir.dt.float32r
BF16 = mybir.dt.bfloat16
I32 = mybir.dt.int32
U32 = mybir.dt.uint32
U16 = mybir.dt.uint16
AF = mybir.ActivationFunctionType
ALU = mybir.AluOpType
AX = mybir.AxisListType

B, T, C, H, T0 = 8, 8192, 128, 8, 16
NP = T // T0          # 512 patches
PATCH = T0 * C        # 2048 elements per patch
S = 65                # sink + 64 selected tokens
NSEL = 64
EPS = 1.1920929e-07
SCALE = 1.0 / float(np.sqrt(np.float32(C)))
NEG_BIG = -1.0e30


def rap(t, apl, offset=0):
    """Raw AP over a tile/AP's storage, flat element strides."""
    base = t if isinstance(t, bass.AP) else t[:]
    return bass.AP(tensor=base.tensor, offset=base.offset + offset,
                   ap=[list(x) for x in apl])


def f32r(ap):
    return ap.bitcast(F32R)


def build_kernel(nc):
    xb = nc.dram_tensor("xb", [T, C], F32, kind="ExternalInput")
    pw = nc.dram_tensor("pw", [1, PATCH + 128], F32R, kind="ExternalInput")
    wqT_d = nc.dram_tensor("wqT_d", [C, 4 * C * H], F32R, kind="ExternalInput")
    woT_d = nc.dram_tensor("woT_d", [C, H, C], F32, kind="ExternalInput")
    cosdup = nc.dram_tensor("cosdup", [128, C], F32, kind="ExternalInput")
    sinpm = nc.dram_tensor("sinpm", [128, C], F32, kind="ExternalInput")
    cmask = nc.dram_tensor("cmask", [S, S], F32, kind="ExternalInput")
    negio = nc.dram_tensor("negio", [1, NP], F32, kind="ExternalInput")
    sinkTq = nc.dram_tensor("sinkTq", [C, H], F32, kind="ExternalInput")
    sinkTk = nc.dram_tensor("sinkTk", [C, H], F32, kind="ExternalInput")
    sinkvb = nc.dram_tensor("sinkvb", [1, H * C], U16, kind="ExternalInput")
    sel16 = nc.dram_tensor("sel16", [5, NSEL], F32, kind="ExternalInput")
    ident = nc.dram_tensor("ident", [128, 128], F32, kind="ExternalInput")
    out = nc.dram_tensor("out", [NSEL, C], F32, kind="ExternalOutput")

    with tile.TileContext(nc) as tc:
        _emit(tc, nc, xb, pw, wqT_d, woT_d, cosdup, sinpm, cmask,
              negio, sinkTq, sinkTk, sinkvb, sel16, ident, out)
    return nc


def _emit(tc, nc, xb, pw, wqT_d, woT_d, cosdup, sinpm, cmask,
          negio, sinkTq, sinkTk, sinkvb, sel16, ident, out):
    import os
    LEVEL = int(os.environ.get("KLEVEL", "9"))
    from contextlib import ExitStack
    ctx = ExitStack()
    with ctx:
        const1 = ctx.enter_context(tc.tile_pool(name="const1", bufs=1))
        xpool = ctx.enter_context(tc.tile_pool(name="xpool", bufs=1))
        junkp = ctx.enter_context(tc.tile_pool(name="junkp", bufs=1))
        stat = ctx.enter_context(tc.tile_pool(name="stat", bufs=4))
        sb = ctx.enter_context(tc.tile_pool(name="sb", bufs=1))
        psall = ctx.enter_context(tc.tile_pool(name="psall", bufs=1,
                                               space="PSUM"))
        # one tile owning all 8 PSUM banks; regions choreographed manually
        PS = psall.tile([128, 4096], F32)
        # region plan (f32 cols):
        #   0:1024     qk stack (q rows 0:64, k rows 64:128); later att
        #              [65, 520] at 0:520; later out [64, 128]
        #   1024:2048  v rows 0:64; later pT staging [65, 260] at 1024:1284
        #   2048:3072  g rows 0:64
        #   2560:3584  y [65, 1024] (after sigg consumed g)
        #   3072:3584  staging A (pw bcast, x_selT, qnT)
        #   3584:4096  logits row (rows 0:1) / knT staging / ygT staging
        LROW = 3584

        # ---------------- pw/ones + ident first, then the x stream --------
        pwo_sb = const1.tile([1, PATCH + 128], F32R)
        nc.sync.dma_start(out=pwo_sb[:, :], in_=pw[:, :])
        pw_sb = pwo_sb[0:1, 0:PATCH]
        ones_t = pwo_sb[0:1, PATCH:PATCH + 128]
        ident_t = const1.tile([128, 128], F32)
        nc.sync.dma_start(out=ident_t[:, :], in_=ident[:, :])

        def xdma(i):
            xp = xpool.tile([128, PATCH], F32, tag=f"xp{i}")
            nc.sync.dma_start(
                out=xp[:, :],
                in_=rap(xb[:, :], [[PATCH, 128], [1, PATCH]],
                        offset=i * 128 * PATCH))
            return xp

        xps = [xdma(0)]
        XSPLIT = True
        eps_t = const1.tile([128, 1], F32)
        nc.vector.memset(eps_t[:, :], EPS)

        # ---------------- remaining x tiles (tile 3 in halves) ----------
        for i in (1, 2):
            xps.append(xdma(i))
        xp3 = xpool.tile([128, PATCH], F32, tag="xp3")
        for hh in range(2):
            nc.sync.dma_start(
                out=xp3[:, 1024 * hh:1024 * (hh + 1)],
                in_=rap(xb[:, :], [[PATCH, 128], [1, 1024]],
                        offset=3 * 128 * PATCH + 1024 * hh))
        xps.append(xp3)
        cosdup_t = const1.tile([128, C], F32)
        nc.sync.dma_start(out=cosdup_t[:, :], in_=cosdup[:, :])
        sinpm_t = const1.tile([128, C], F32)
        nc.sync.dma_start(out=sinpm_t[:, :], in_=sinpm[:, :])
        cmask_t = const1.tile([S, S], F32)
        nc.sync.dma_start(out=cmask_t[:, :], in_=cmask[:, :])
        negio_t = const1.tile([1, NP], F32)
        nc.sync.dma_start(out=negio_t[:, :], in_=negio[:, :])
        sinkTq_t = const1.tile([C, H], F32)
        nc.sync.dma_start(out=sinkTq_t[:, :], in_=sinkTq[:, :])
        sinkTk_t = const1.tile([C, H], F32)
        nc.sync.dma_start(out=sinkTk_t[:, :], in_=sinkTk[:, :])
        sel16_t = const1.tile([5, NSEL], F32)
        nc.sync.dma_start(out=sel16_t[:, :], in_=sel16[:, :])
        rhs5 = const1.tile([5, 1], F32)
        nc.vector.memset(rhs5[:, :], 1.0)

        # v sink row: host-rounded bf16 bits straight into v_sb row 64
        v_sb = sb.tile([S, H, C], BF16, tag="v_sb")
        nc.sync.dma_start(
            out=v_sb[NSEL:S, :, :],
            in_=sinkvb[:, :].bitcast(BF16).rearrange(
                "p (h c) -> p h c", h=H))

        wqT = const1.tile([C, 4 * C * H], F32R)
        for wch in range(4):
            nc.sync.dma_start(out=wqT[:, 1024 * wch:1024 * (wch + 1)],
                              in_=wqT_d[:, 1024 * wch:1024 * (wch + 1)])
        woT = const1.tile([C, H, C], F32)


        # preload the sqrt activation table while ACT is idle
        dummy = stat.tile([1, 1], F32)
        nc.vector.memset(dummy[:, :], 1.0)
        nc.scalar.activation(out=dummy[:, :], in_=dummy[:, :],
                             func=AF.Sqrt)

        woTb = const1.tile([C, H, C], BF16)

        # pw broadcast to 128 partitions via K=1 matmul into staging banks
        # (two tiny warmups first lift PE off the cold p-state)
        nc.tensor.matmul(out=PS[0:128, 3071:3072], lhsT=ident_t[:, :],
                         rhs=ident_t[:, 0:1], start=True, stop=True)
        nc.tensor.matmul(out=PS[0:128, 3071:3072], lhsT=ident_t[:, :],
                         rhs=ident_t[:, 0:1], start=True, stop=True)
        pwB = const1.tile([128, PATCH], F32)
        for q4 in range(4):
            base = 1536 + 512 * q4
            nc.tensor.matmul(out=PS[:, base:base + 512],
                             lhsT=ones_t,
                             rhs=pwo_sb[0:1, 512 * q4:512 * (q4 + 1)],
                             start=True, stop=True)
            if q4 % 2 == 0:
                nc.scalar.copy(out=pwB[:, 512 * q4:512 * (q4 + 1)],
                               in_=PS[:, base:base + 512])
            else:
                nc.vector.tensor_copy(out=pwB[:, 512 * q4:512 * (q4 + 1)],
                                      in_=PS[:, base:base + 512])

        # ---------------- phase 1: per-patch stats ----------------
        junk = junkp.tile([128, PATCH], F32, tag="junk")
        junk2 = junkp.tile([128, PATCH], F32, tag="junk2")
        ss_c = stat.tile([128, 4], F32, tag="ss_c")
        dot_c = stat.tile([128, 4], F32, tag="dot_c")
        rs_c = stat.tile([128, 4], F32, tag="rs_c")
        logit_c = stat.tile([128, 4], F32, tag="logit_c")
        ss_h = stat.tile([128, 2], F32, tag="ss_h")
        dot_h = stat.tile([128, 2], F32, tag="dot_h")
        for i in range(4):
            xp = xps[i]
            if i < 3:
                nc.scalar.activation(out=junk[:, :], in_=xp[:, :],
                                     func=AF.Square,
                                     accum_out=ss_c[:, i:i + 1])
                nc.vector.scalar_tensor_tensor(
                    out=junk2[:, :], in0=xp[:, :], scalar=1.0, in1=pwB[:, :],
                    op0=ALU.mult, op1=ALU.mult,
                    accum_out=dot_c[:, i:i + 1])
            else:
                # tile 3 in halves so its stats overlap its own DMA
                for hh in range(2):
                    cs = slice(1024 * hh, 1024 * (hh + 1))
                    nc.scalar.activation(out=junk[:, cs], in_=xp[:, cs],
                                         func=AF.Square,
                                         accum_out=ss_h[:, hh:hh + 1])
                    nc.vector.scalar_tensor_tensor(
                        out=junk2[:, cs], in0=xp[:, cs], scalar=1.0,
                        in1=pwB[:, cs], op0=ALU.mult, op1=ALU.mult,
                        accum_out=dot_h[:, hh:hh + 1])
                nc.scalar.activation(out=ss_c[:, 3:4], in_=ss_h[:, 0:1],
                                     func=AF.Identity, bias=ss_h[:, 1:2])
                nc.vector.tensor_add(out=dot_c[:, 3:4], in0=dot_h[:, 0:1],
                                     in1=dot_h[:, 1:2])
            nc.scalar.activation(out=rs_c[:, i:i + 1], in_=ss_c[:, i:i + 1],
                                 func=AF.Sqrt, bias=eps_t[:, :],
                                 scale=1.0 / PATCH)
            nc.vector.reciprocal(out=rs_c[:, i:i + 1], in_=rs_c[:, i:i + 1])
            nc.vector.tensor_mul(logit_c[:, i:i + 1], dot_c[:, i:i + 1],
                                 rs_c[:, i:i + 1])
            # transpose this tile's logits column into the PSUM row
            nc.tensor.transpose(
                out=PS[0:1, LROW + 128 * i:LROW + 128 * (i + 1)],
                in_=logit_c[:, i:i + 1], identity=ident_t[:, :])

        if LEVEL == 1:
            lrow_sb = stat.tile([1, NP], F32, tag="lrow_sb")
            nc.vector.tensor_copy(out=lrow_sb[:, :],
                                  in_=PS[0:1, LROW:LROW + NP])
            for r in range(4):
                nc.sync.dma_start(out=out[r:r + 1, :],
                                  in_=lrow_sb[0:1, 128 * r:128 * (r + 1)])
            return

        # ---------------- top-4 selection (on the PSUM row) ----------------
        lrow = PS[0:1, LROW:LROW + NP]
        max8 = stat.tile([1, 8], F32, tag="max8")
        nc.vector.max(out=max8[:, :], in_=lrow)
        masked = stat.tile([1, NP], F32, tag="masked")
        nc.vector.scalar_tensor_tensor(
            out=masked[:, :], in0=lrow, scalar=max8[:, 3:4],
            in1=negio_t[:, :], op0=ALU.is_ge, op1=ALU.mult)
        mm8 = stat.tile([1, 8], F32, tag="mm8")
        nc.vector.max(out=mm8[:, :], in_=masked[:, :])
        idx8 = stat.tile([1, 8], U32, tag="idx8")
        nc.vector.max_index(out=idx8[:, :], in_max=mm8[:, :],
                            in_values=masked[:, :])
        idxf = stat.tile([1, 8], F32, tag="idxf")
        nc.vector.tensor_copy(out=idxf[:, :], in_=idx8[:, :])

        # patch-id column via PE transpose: [1,4] -> [4,1], then token
        # ids 16*I[p] + 4T + t in (T, p, t) row order via sel16
        nc.tensor.transpose(out=PS[0:4, 3582:3583], in_=idxf[0:1, 0:4],
                            identity=ident_t[0:1, 0:1])
        nc.scalar.copy(out=rhs5[0:4, :], in_=PS[0:4, 3582:3583])
        nc.tensor.matmul(out=PS[0:NSEL, 3583:3584], lhsT=sel16_t[:, :],
                         rhs=rhs5[:, :], start=True, stop=True)
        idc_i = stat.tile([NSEL, 1], I32, tag="idc_i")
        nc.vector.tensor_copy(out=idc_i[:, :], in_=PS[0:NSEL, 3583:3584])

        # gather the 64 tokens (row 16T+4p+t = token 16*I[p] + 4T + t)
        x_sel = sb.tile([NSEL, C], F32, tag="x_sel")
        nc.gpsimd.indirect_dma_start(
            out=x_sel[:, :], out_offset=None, in_=xb[:, :],
            in_offset=bass.IndirectOffsetOnAxis(ap=idc_i[:, 0:1], axis=0))


        if LEVEL == 2:
            nc.sync.dma_start(out=out[:, :], in_=x_sel[:, :])
            return

        # ---------------- qkvg projection ----------------
        nc.tensor.transpose(out=PS[0:128, 3072:3072 + NSEL], in_=x_sel[:, :],
                            identity=ident_t[0:NSEL, 0:NSEL])
        x_selT = sb.tile([C, NSEL], F32R, tag="x_selT")
        nc.scalar.copy(out=x_selT[:, :], in_=PS[:, 3072:3072 + NSEL])

        # qkvg[token, f] for the 64 gathered tokens -> PS rows 0:64
        for g in range(8):
            nc.tensor.matmul(out=PS[0:NSEL, 512 * g:512 * (g + 1)],
                             lhsT=x_selT[:, :],
                             rhs=wqT[:, 512 * g:512 * (g + 1)],
                             start=True, stop=True)
        qkvg_sb = sb.tile([NSEL, 4 * C * H], BF16, tag="qkvg_sb")
        nc.scalar.copy(out=qkvg_sb[:, 0:1024], in_=PS[0:NSEL, 0:1024])
        nc.vector.tensor_copy(out=qkvg_sb[:, 1024:2048],
                              in_=PS[0:NSEL, 1024:2048])
        nc.scalar.copy(out=qkvg_sb[:, 2048:3072], in_=PS[0:NSEL, 2048:3072])
        nc.vector.tensor_copy(out=qkvg_sb[:, 3072:4096],
                              in_=PS[0:NSEL, 3072:4096])

        # rearrange token-major -> s-major via SBUF->SBUF DMAs.
        # qkvg row 16T+4p+t (token 16*I[p]+4T+t), col (b,h,c) feeds
        # s-row 16p+4t+b of tensor T: per tensor the source rows are the
        # contiguous block 16T:16T+16 -> clean single-stride APs.
        qk = sb.tile([128, H, C], BF16, tag="qk")
        vg = sb.tile([128, H, C], BF16, tag="vg")
        FQ = 4 * C * H

        def rearr(tens, dst, half, eng):
            eng.dma_start(
                out=dst[64 * half:64 * half + NSEL, :, :],
                in_=rap(qkvg_sb[:, :], [[FQ, T0], [1024, 4], [1, 1024]],
                        offset=T0 * tens * FQ))

        rearr(0, qk, 0, nc.sync)    # q on the SP queue
        rearr(1, qk, 1, nc.scalar)  # k on the ACT queue (packs transfers)

        # out-projection weights arrive late; the dummy write makes the DMA
        # wait for the gather so it cannot block the gather's transfer
        nc.vector.tensor_copy(out=woT[0:1, 0, 0:1], in_=x_sel[0:1, 0:1])
        nc.sync.dma_start(out=woT[:, :, :], in_=woT_d[:, :, :])
        nc.gpsimd.tensor_copy(out=woTb[:, :, :], in_=woT[:, :, :])

        if LEVEL == 3:
            q0 = sb.tile([NSEL, C], F32, tag="q0dbg")
            nc.vector.tensor_copy(out=q0[:, :], in_=qk[0:NSEL, 0, :])
            nc.sync.dma_start(out=out[:, :], in_=q0[:, :])
            return

        # ---------------- rmsnorm + rope on the qk stack ----------------
        ssq = sb.tile([128, H], F32, tag="ssq")
        sqj = junkp.tile([128, H, C], F32, tag="sqj")
        nc.gpsimd.tensor_tensor(out=sqj[:, 5:8, :], in0=qk[:, 5:8, :],
                                in1=qk[:, 5:8, :], op=ALU.mult)
        nc.vector.tensor_tensor(out=sqj[:, 0:5, :], in0=qk[:, 0:5, :],
                                in1=qk[:, 0:5, :], op=ALU.mult)
        # gate the v/g rearranges on the stt output so the scheduler keeps
        # sigmoid's table load behind the rope sqrt (value-preserving
        # corner write on the v/g source rows)
        zro = stat.tile([17, 1], F32, tag="zro")
        nc.vector.tensor_scalar_mul(zro[:, :], sqj[0:17, 0, 0:1], 0.0)
        corner = rap(qkvg_sb[:, :], [[FQ, 17], [1, 1]], offset=32 * FQ)
        nc.vector.tensor_scalar(out=corner, in0=corner,
                                scalar1=zro[:, 0:1], scalar2=None,
                                op0=ALU.add)
        rearr(2, vg, 0, nc.sync)
        rearr(3, vg, 1, nc.sync)
        nc.gpsimd.tensor_copy(out=v_sb[0:NSEL, :, :], in_=vg[0:NSEL, :, :])
        nc.vector.tensor_reduce(out=ssq[:, 0:5], in_=sqj[:, 0:5, :],
                                axis=AX.X, op=ALU.add)
        nc.vector.tensor_reduce(out=ssq[:, 5:8], in_=sqj[:, 5:8, :],
                                axis=AX.X, op=ALU.add)
        rf = sb.tile([128, H], F32, tag="rf")
        nc.scalar.activation(out=rf[:, :], in_=ssq[:, :], func=AF.Sqrt,
                             bias=eps_t[:, :], scale=1.0 / C)
        nc.vector.reciprocal(out=rf[:, :], in_=rf[:, :])
        # sigmoid gate now, exp loads after: both ACT table loads land in
        # the rope shadow, and the softmax exps then run load-free
        sigg = sb.tile([NSEL, H, C], BF16, tag="sigg")
        nc.scalar.activation(out=sigg[:, :, :], in_=vg[NSEL:128, :, :],
                             func=AF.Sigmoid)
        qk1 = sb.tile([128, H, C], F32, tag="qk1")
        r1 = sb.tile([128, H, C], F32, tag="r1")
        r2 = sb.tile([128, H, C], F32, tag="r2")
        qkn = sb.tile([128, H, C], F32, tag="qkn")

        def hs(eng, hs0, hs1):
            hn = hs1 - hs0
            eng.tensor_tensor(
                out=qk1[:, hs0:hs1, :], in0=qk[:, hs0:hs1, :],
                in1=rf[:, hs0:hs1].rearrange("p (h a) -> p h a", a=1)
                    .to_broadcast([128, hn, C]), op=ALU.mult)
            eng.tensor_tensor(
                out=r1[:, hs0:hs1, :], in0=qk1[:, hs0:hs1, :],
                in1=cosdup_t[:, :].rearrange("p (a c) -> p a c", a=1)
                    .to_broadcast([128, hn, C]), op=ALU.mult)
            eng.tensor_tensor(
                out=r2[:, hs0:hs1, 0:64], in0=qk1[:, hs0:hs1, 64:128],
                in1=sinpm_t[:, 0:64].rearrange("p (a c) -> p a c", a=1)
                    .to_broadcast([128, hn, 64]), op=ALU.mult)
            eng.tensor_tensor(
                out=r2[:, hs0:hs1, 64:128], in0=qk1[:, hs0:hs1, 0:64],
                in1=sinpm_t[:, 64:128].rearrange("p (a c) -> p a c", a=1)
                    .to_broadcast([128, hn, 64]), op=ALU.mult)
            eng.tensor_add(out=qkn[:, hs0:hs1, :], in0=r1[:, hs0:hs1, :],
                           in1=r2[:, hs0:hs1, :])

        hs(nc.vector, 0, 5)
        hs(nc.gpsimd, 5, 8)

        if LEVEL == 4:
            qn32 = sb.tile([NSEL, C], F32, tag="qn32")
            nc.vector.tensor_copy(out=qn32[:, :], in_=qkn[0:NSEL, 0, :])
            nc.sync.dma_start(out=out[:, :], in_=qn32[:, :])
            return

        # ---------------- transposes to qnT / knT ----------------
        # per head-group so group-0 attention starts while group-1 is
        # still transposing; sink columns inserted up front
        qnT = sb.tile([C, H, S], BF16, tag="qnT")
        knT = sb.tile([C, H, S], BF16, tag="knT")
        nc.scalar.copy(out=rap(qnT[:, :, :], [[H * S, C], [S, H], [1, 1]],
                               offset=NSEL),
                       in_=sinkTq_t[:, :].rearrange("c (h a) -> c h a", a=1))
        nc.scalar.copy(out=rap(knT[:, :, :], [[H * S, C], [S, H], [1, 1]],
                               offset=NSEL),
                       in_=sinkTk_t[:, :].rearrange("c (h a) -> c h a", a=1))
        for g in range(2):
            for si, dstT in enumerate((qnT, knT)):
                base = 3072 + 256 * (2 * g + si)
                for j in range(4):
                    h = 4 * g + j
                    nc.tensor.transpose(
                        out=PS[:, base + NSEL * j:base + NSEL * (j + 1)],
                        in_=qkn[64 * si:64 * (si + 1), h, :],
                        identity=ident_t[64 * si:64 * si + NSEL,
                                         64 * si:64 * si + NSEL])
                dst = rap(dstT[:, :, :], [[H * S, C], [S, 4], [1, NSEL]],
                          offset=4 * g * S)
                nc.vector.tensor_copy(
                    out=dst, in_=PS[:, base:base + 256].rearrange(
                        "p (h s) -> p h s", h=4))

        # ---------------- attention ----------------
        # att head slots padded to 128 cols (matmul must not cross banks);
        # the whole tail runs as two independent head-group pipelines so
        # PE/DVE/ACT overlap across groups
        t0 = sb.tile([S, H, S], F32, tag="t0")
        p_sb = sb.tile([S, H, S], F32, tag="p_sb")
        den8 = sb.tile([S, H], F32, tag="den8")
        rden = sb.tile([S, H], F32, tag="rden")
        sigrd = sb.tile([NSEL, H, C], F32, tag="sigrd")
        pT = sb.tile([S, H, S], BF16, tag="pT")
        ygT = sb.tile([C, H, NSEL], BF16, tag="ygT")
        sgT_sb = sb.tile([C, H, NSEL], BF16, tag="sgT_sb")
        for g in range(2):
            hs = slice(4 * g, 4 * (g + 1))
            for h in range(4 * g, 4 * (g + 1)):
                nc.tensor.matmul(out=PS[0:S, C * h:C * h + S],
                                 lhsT=qnT[:, h, :], rhs=knT[:, h, :],
                                 start=True, stop=True)
            attg = rap(PS[:, :], [[4096, S], [C, 4], [1, S]],
                       offset=4 * g * C)
            nc.vector.tensor_tensor(
                out=t0[:, hs, :], in0=attg,
                in1=cmask_t[:, :].rearrange("s (a t) -> s a t", a=1)
                    .to_broadcast([S, 4, S]), op=ALU.add)
            nc.scalar.activation(out=p_sb[:, hs, :], in_=t0[:, hs, :],
                                 func=AF.Exp, scale=SCALE)
            nc.vector.tensor_reduce(out=den8[:, hs], in_=p_sb[:, hs, :],
                                    axis=AX.X, op=ALU.add)
            nc.vector.reciprocal(out=rden[:, hs], in_=den8[:, hs])
            # 1/den folds into the gate; pT/y consume UNNORMALIZED p
            nc.vector.tensor_tensor(
                out=sigrd[:, hs, :], in0=sigg[:, hs, :],
                in1=rden[0:NSEL, hs].rearrange("s (h a) -> s h a", a=1)
                    .to_broadcast([NSEL, 4, C]), op=ALU.mult)
            for j in range(4):
                nc.tensor.transpose(
                    out=PS[0:S, 1024 + 520 * g + S * j:
                           1024 + 520 * g + S * (j + 1)],
                    in_=p_sb[:, 4 * g + j, :], identity=ident_t[0:S, 0:S])
            nc.scalar.copy(
                out=pT[:, hs, :],
                in_=PS[0:S, 1024 + 520 * g:1024 + 520 * g + 4 * S]
                    .rearrange("p (a b) -> p a b", a=4))
            # yT = v^T @ p per head (swapped operands) -> [c, s] slots
            for h in range(4 * g, 4 * (g + 1)):
                nc.tensor.matmul(out=PS[0:C, 2560 + C * h:2560 + C * h + S],
                                 lhsT=v_sb[:, h, :], rhs=pT[:, h, :],
                                 start=True, stop=True)
            # transpose the gate into [c, h, s] during the same window
            for h in range(4 * g, 4 * (g + 1)):
                nc.tensor.transpose(
                    out=PS[:, LROW + NSEL * h:LROW + NSEL * (h + 1)],
                    in_=sigrd[:, h, :], identity=ident_t[0:NSEL, 0:NSEL])
            nc.scalar.copy(
                out=rap(sgT_sb[:, :, :],
                        [[H * NSEL, C], [NSEL, 4], [1, NSEL]],
                        offset=4 * g * NSEL),
                in_=PS[:, LROW + 256 * g:LROW + 256 * (g + 1)].rearrange(
                    "p (h s) -> p h s", h=4))
            yTg = rap(PS[:, :], [[4096, C], [C, 4], [1, NSEL]],
                      offset=2560 + 4 * g * C)
            nc.vector.tensor_tensor(
                out=rap(ygT[:, :, :], [[H * NSEL, C], [NSEL, 4], [1, NSEL]],
                        offset=4 * g * NSEL),
                in0=yTg,
                in1=rap(sgT_sb[:, :, :],
                        [[H * NSEL, C], [NSEL, 4], [1, NSEL]],
                        offset=4 * g * NSEL), op=ALU.mult)

        if LEVEL == 5:
            yg32 = sb.tile([NSEL, C], F32, tag="yg32")
            nc.vector.tensor_copy(out=yg32[:, :], in_=ygT[0:NSEL, 0, :])
            nc.sync.dma_start(out=out[:, :], in_=yg32[:, :])
            return

        # ---------------- output projection ----------------
        out_sb = sb.tile([NSEL, C], F32, tag="out_sb")
        for half in range(2):
            cols = slice(64 * half, 64 * (half + 1))
            out_ps = PS[0:NSEL, 64 * half:64 * (half + 1)]
            for h in range(H):
                nc.tensor.matmul(out=out_ps, lhsT=ygT[:, h, :],
                                 rhs=woTb[:, h, cols], start=(h == 0),
                                 stop=(h == H - 1))
            nc.scalar.copy(out=out_sb[:, cols], in_=out_ps)
            nc.sync.dma_start(out=out[:, cols], in_=out_sb[:, cols])


def make_host_constants(inputs):
    """Host-side prep of tables derived from the (full) inputs."""
    cos = np.asarray(inputs["cos"]).reshape(S, 64).astype(np.float32)
    sin = np.asarray(inputs["sin"]).reshape(S, 64).astype(np.float32)
    sink = np.asarray(inputs["sink"]).reshape(H, C).astype(np.float32)
    tao = np.asarray(inputs["tao"]).astype(np.float32)
    wq = np.asarray(inputs["W_qkvg"]).astype(np.float32)
    wo = np.asarray(inputs["W_out"]).astype(np.float32)

    # partition p (0..63 in each half) holds position p+1; rows duplicated
    # for the q half (0:64) and k half (64:128)
    pos = np.arange(64) + 1
    cos_p = cos[pos]
    sin_p = sin[pos]
    cosdup = np.tile(np.concatenate([cos_p, cos_p], axis=1), (2, 1))
    sinpm = np.tile(np.concatenate([sin_p, -sin_p], axis=1), (2, 1))
    # tao folds into the rope tables: qn = (qk*rf)*cos' + swap(qk*rf)*sin'
    taocol = np.concatenate([np.full((64, 1), tao[0], np.float32),
                             np.full((64, 1), tao[1], np.float32)])
    cosdup = cosdup * taocol
    sinpm = sinpm * taocol

    # additive causal mask in s-major layout (row/col 64 = sink, pos 0)
    posf = np.where(np.arange(S) < NSEL, np.arange(S) + 1, 0)
    cmaskm = np.where(posf[None, :] <= posf[:, None], 0.0,
                      NEG_BIG).astype(np.float32)
    negio = (float(NP) - np.arange(NP, dtype=np.float32)).reshape(1, NP)

    # sink rows: rope at position 0 is identity; rmsnorm + tao on host
    sn = sink / np.sqrt((sink * sink).mean(axis=-1, keepdims=True) + EPS)
    sinkTq = np.ascontiguousarray((sn * tao[0]).T)
    sinkTk = np.ascontiguousarray((sn * tao[1]).T)
    # v sink row as bf16 bit pattern (round-to-nearest-even)
    f = sink.reshape(1, H * C).astype(np.float32)
    u = f.view(np.uint32)
    rounded = ((u + 0x7FFF + ((u >> 16) & 1)) >> 16).astype(np.uint16)
    sinkvb = np.ascontiguousarray(rounded)

    # token ids: row 16T+4p+t gathers token 16*I[p] + 4T + t
    # sel16[j, r] = 16*(j==p(r)) for j<4; sel16[4, r] = 4T(r) + t(r)
    sel16m = np.zeros((5, NSEL), np.float32)
    for Tn in range(4):
        for p in range(4):
            for t in range(4):
                r = 16 * Tn + 4 * p + t
                sel16m[p, r] = 16.0
                sel16m[4, r] = float(4 * Tn + t)

    wqT = np.ascontiguousarray(wq.T)
    woT = np.ascontiguousarray(wo.reshape(C, H, C).transpose(2, 1, 0))

    ident = np.eye(128, dtype=np.float32)
    return dict(cosdup=np.ascontiguousarray(cosdup, dtype=np.float32),
                sinpm=np.ascontiguousarray(sinpm, dtype=np.float32),
                cmask=cmaskm, negio=negio,
                sinkTq=sinkTq, sinkTk=sinkTk, sinkvb=sinkvb, sel16=sel16m,
                wqT_d=wqT, woT_d=woT, ident=ident)


_CACHE = {}


def get_nc():
    if "nc" not in _CACHE:
        nc = bacc.Bacc("TRN2", target_bir_lowering=False, debug=False,
                       num_devices=B)
        build_kernel(nc)
        nc.compile()
        _CACHE["nc"] = nc
    return _CACHE["nc"]


def make_in_maps(inputs):
    x = np.ascontiguousarray(inputs["x"], dtype=np.float32)
    pwv = np.concatenate(
        [np.asarray(inputs["patch_w"], np.float32).ravel(),
         np.ones(128, np.float32)]).reshape(1, PATCH + 128)
    consts = make_host_constants(inputs)
    in_maps = []
    for b in range(B):
        m = {"xb": np.ascontiguousarray(x[b]), "pw": pwv}
        m.update(consts)
        in_maps.append(m)
    return in_maps


def kernel(**inputs):
    nc = get_nc()
    in_maps = make_in_maps(inputs)
    res = run_bass_kernel_spmd(nc, in_maps, core_ids=list(range(B)))
    return np.stack([r["out"] for r in res.results], axis=0)


if __name__ == "__main__":
    nc = get_nc()
    print("build ok:", len(nc.m.functions[0].allocations), "allocations")
